# revision 39
# baseline (speedup 1.0000x reference)
"""nn_GroupAttention Trainium2 kernel (8-core SPMD), v3.

Sharding: core = (b, s): b = core//2 (batch), s = core%2 (head half).
Each core handles batch b, heads 8s..8s+7 = d_model channels 512s..512s+512
(= groups 2s, 2s+1).  Device tensors are channel-major ("transposed") so all
matmul contractions run over the partition dim.

v3 structure:
- kv LayerNorm is folded into the K/V projections as post-matmul fixups
  (K = (Wk'x)*rho - ksum*mrh + kbeta, V likewise with transposed per-token
  stats), so the K/V matmuls depend only on the raw loads and the serial
  in-place normalize leaves the critical path entirely.
- The transformer-XL rel-shift DRAM bounce for ALL 8 heads is issued right
  after the q/r projections, landing in per-head SBUF scratch tiles; the
  attention inner loop has no DMA dependencies (keeps the PE HAM-warm).
- Scalar-engine activations are batched by function set (squares, then
  ln/exp stats, then copies, then softmax exps) to avoid ACT table reloads.
- avT transposes + inter-projection partial sums run inside the head-pair
  loop; the pair AllReduce (bf16 payload) is issued immediately after the
  last pair and overlaps the intra matmuls.  A dummy early collective
  absorbs the collective firmware entry cost.
"""
import sys

sys.path.insert(0, "/opt/trn_rl_repo")
import numpy as np

Q, M, KLEN, B = 512, 512, 1024, 4
D, H, DH, G = 1024, 16, 64, 4
DG = D // G
EPS = 1e-6
SCALE = 0.125
VMASK = -1e5
WBLK = 1152                 # uniform DRAM/staging block width per query tile
SOFF = [0, 640, 1408, 2304]  # packed scratch offsets (widths 640/768/896/1024)
VALID = {it: [jt for jt in range(8) if jt - it <= 4] for it in range(4)}

_cache = {}


def _build_nc(dump=False):
    import concourse.bass as bass
    import concourse.bacc as bacc
    import concourse.mybir as mybir
    import concourse.tile as tile

    BF = mybir.dt.bfloat16
    F32 = mybir.dt.float32
    AF = mybir.ActivationFunctionType
    ALU = mybir.AluOpType

    nc = bacc.Bacc("TRN2", target_bir_lowering=False, debug=False,
                   num_devices=8)

    def din(name, shape, dt=BF):
        return nc.declare_dram_parameter(name, list(shape), dt, isOutput=False)

    kvT = din("kvT", [D, KLEN])
    wTp = din("wTp", [D, Q])              # channel-PERMUTED w^T
    wres = din("wres", [512, Q], mybir.dt.float32)
    rT = din("rT", [512, KLEN])
    WkT = din("WkT", [D, 512])
    WvT = din("WvT", [D, 512])
    WiqT = din("WiqT", [D, DG])           # row-permuted to match wTp
    WqT = din("WqT", [2, DG, DG])
    WrT = din("WrT", [2, DG, DG])
    WintraT = din("WintraT", [2, DG, DG])
    WinterT = din("WinterT", [512, DG])
    kbeta = din("kbeta", [128, 4], F32)
    ksum = din("ksum", [128, 4], F32)
    vsum = din("vsum", [128, 512])        # row-replicated -Wv' row sums (bf16)
    rwb = din("rwb", [128, 4], F32)
    rrb = din("rrb", [128, 4], F32)
    ident = din("ident", [128, 128])
    out = nc.declare_dram_parameter("out", [512, Q], F32, isOutput=True)
    dmp = (nc.declare_dram_parameter("dmp", [128, 12 * 1024], F32, isOutput=True)
           if dump else None)

    def dump_tile(k, ap):
        if dmp is not None:
            nc.gpsimd.dma_start(dmp[:, k * 1024:k * 1024 + ap.shape[-1]], ap)

    bd_dram = [nc.dram_tensor(f"bd_dram{h}", [128, 4 * WBLK], BF)
               for h in range(8)]
    cc_in = nc.dram_tensor("cc_in", [DG, Q], BF)
    cc_out = nc.dram_tensor("cc_out", [DG, Q], BF)
    ccd_in = nc.dram_tensor("ccd_in", [128, 8], BF)
    ccd_out = nc.dram_tensor("ccd_out", [128, 8], BF)

    with tile.TileContext(nc) as tc:
        with (
            tc.tile_pool(name="persist", bufs=1) as P,
            tc.tile_pool(name="work", bufs=2) as W,
        ):
            ones_sq = P.tile([128, 128], BF, tag="onessq")
            nc.gpsimd.memset(ones_sq[:], 1.0)
            id_sb = P.tile([128, 128], BF, tag="id")
            nc.sync.dma_start(id_sb[:], ident[:])

            KT_sb = P.tile([128, 4 * KLEN], BF, tag="KT")       # dt-major
            V_sb = P.tile([128, 8 * 520], BF, tag="V")          # jt x (h x 65)
            QbT = P.tile([128, 4 * Q], BF, tag="QbT")
            QcT = P.tile([128, 4 * Q], BF, tag="QcT")
            rhT = P.tile([128, 4 * KLEN], BF, tag="rhT")        # dt-major
            attn_sb = P.tile([128, 4 * 512], BF, tag="attn")    # it-major
            avT_sb = P.tile([128, 4 * 512], BF, tag="avT")
            Wintra_sb = P.tile([128, 4 * DG], BF, tag="wintra")
            Winter_sb = P.tile([128, 4 * DG], BF, tag="winter")

            # V softmax-denominator ones columns (cols 64 mod 65), static
            for jt in range(8):
                nc.gpsimd.memset(
                    bass.AP(V_sb[:].tensor, V_sb[:].offset + jt * 520 + 64,
                            [[V_sb[:].ap[0][0], 128], [65, 8], [1, 1]]), 1.0)

            # per-head shifted-BD scratch, packed query-tile blocks
            SCR = tc.tile_pool(name="scratch", bufs=1)
            SCRc = SCR.__enter__()
            scratch = [SCRc.tile([128, 3328], BF, tag=f"scr{h}", name=f"scr{h}")
                       for h in range(8)]

            # ================= phase A: stats =================
            SPAN = tc.tile_pool(name="span_kv", bufs=1)
            SPANc = SPAN.__enter__()
            xhat = SPANc.tile([128, 8 * KLEN], BF, tag="xhat")   # RAW kv
            SPANW = tc.tile_pool(name="span_w", bufs=1)
            SPANWc = SPANW.__enter__()
            xwhat = SPANWc.tile([128, 8 * Q], BF, tag="xwhat")
            LNS = tc.tile_pool(name="lnS", bufs=1)
            LNSc = LNS.__enter__()
            rho_bt = LNSc.tile([128, KLEN], BF, tag="rhobt")
            mrh_bt = LNSc.tile([128, KLEN], BF, tag="mrhbt")
            rv_sb = LNSc.tile([128, 16], F32, tag="rvsb")   # rho^T | mrh^T per jt
            with (
                tc.tile_pool(name="lnAa", bufs=1) as LAWa,
                tc.tile_pool(name="lnAw", bufs=2) as LAW,
                tc.tile_pool(name="psA", bufs=1, space="PSUM") as PSA,
            ):
                for ct in range(8):
                    nc.sync.dma_start(
                        xwhat[:, ct * Q:(ct + 1) * Q],
                        wTp[ct * 128:(ct + 1) * 128, :])
                for ct in range(8):
                    nc.sync.dma_start(
                        xhat[:, ct * KLEN:(ct + 1) * KLEN],
                        kvT[ct * 128:(ct + 1) * 128, :])

                def stats_bt(ps_sum_ap, ps_sq_ap, n, width, rho_out, mrh_out, nm):
                    mu = LAWa.tile([128, width], F32, tag="mu", name=f"mu{nm}")
                    t1 = LAWa.tile([128, width], F32, tag="t1", name=f"t1{nm}")
                    va = LAWa.tile([128, width], F32, tag="va", name=f"va{nm}")
                    nc.vector.tensor_scalar_mul(mu[:], ps_sum_ap, 1.0 / n)
                    nc.vector.tensor_mul(t1[:], mu[:], mu[:])
                    nc.vector.tensor_scalar_mul(va[:], ps_sq_ap, 1.0 / (n - 1))
                    nc.vector.scalar_tensor_tensor(
                        va[:], t1[:], -float(n) / (n - 1), va[:],
                        op0=ALU.mult, op1=ALU.add)
                    # 1/(sigma+eps) ~= 1/sigma = exp(-0.5*ln(var)); eps err ~1e-6
                    nc.scalar.activation(t1[:], va[:], AF.Ln)
                    nc.scalar.activation(rho_out, t1[:], AF.Exp, scale=-0.5)
                    nc.vector.tensor_mul(mrh_out, mu[:], rho_out)

                rhow_bt = LAWa.tile([128, 4 * Q], BF, tag="rhowbt")
                mrhw_bt = LAWa.tile([128, 4 * Q], BF, tag="mrhwbt")

                # all Square activations first (one ACT table set)
                sqw_tiles = []
                for g in range(4):
                    for kt in range(2):
                        ct = 2 * g + kt
                        sqw = LAW.tile([128, Q], BF, tag="sqw", name=f"sqw{ct}")
                        nc.scalar.activation(sqw[:], xwhat[:, ct * Q:(ct + 1) * Q],
                                             AF.Square)
                        sqw_tiles.append(sqw)
                # w-group stats matmuls
                ps_w = []
                for g in range(4):
                    ps_gs = PSA.tile([128, Q], F32, tag="psg", bufs=4,
                                     name=f"psgs{g}")
                    ps_gq = PSA.tile([128, Q], F32, tag="psg", bufs=4,
                                     name=f"psgq{g}")
                    for kt in range(2):
                        ct = 2 * g + kt
                        nc.tensor.matmul(ps_gs[:], ones_sq[:],
                                         xwhat[:, ct * Q:(ct + 1) * Q],
                                         start=(kt == 0), stop=(kt == 1))
                        nc.tensor.matmul(ps_gq[:], ones_sq[:], sqw_tiles[ct][:],
                                         start=(kt == 0), stop=(kt == 1))
                    ps_w.append((ps_gs, ps_gq))

                # kv stats: squares + all-ones matmuls
                ps_sum = PSA.tile([128, KLEN], F32, tag="pssum")
                ps_sq = PSA.tile([128, KLEN], F32, tag="pssq")
                for ct in range(8):
                    sq = LAW.tile([128, KLEN], BF, tag="sq")
                    nc.scalar.activation(sq[:], xhat[:, ct * KLEN:(ct + 1) * KLEN],
                                         AF.Square)
                    for nb in range(2):
                        sl = slice(nb * 512, (nb + 1) * 512)
                        nc.tensor.matmul(
                            ps_sum[:, sl], ones_sq[:],
                            xhat[:, ct * KLEN + nb * 512:ct * KLEN + (nb + 1) * 512],
                            start=(ct == 0), stop=(ct == 7))
                        nc.tensor.matmul(ps_sq[:, sl], ones_sq[:], sq[:, sl],
                                         start=(ct == 0), stop=(ct == 7))

                # stats (ln/exp batched after all squares)
                for g in range(4):
                    gsl = slice(g * Q, (g + 1) * Q)
                    stats_bt(ps_w[g][0][:], ps_w[g][1][:], DG, Q,
                             rhow_bt[:, gsl], mrhw_bt[:, gsl], f"w{g}")
                stats_bt(ps_sum[:], ps_sq[:], D, KLEN, rho_bt[:], mrh_bt[:], "kv")

                # normalize xwhat in place (small; q projections consume it)
                for ct in range(8):
                    g = ct // 2
                    sl = slice(ct * Q, (ct + 1) * Q)
                    gsl = slice(g * Q, (g + 1) * Q)
                    eng = nc.vector if ct % 2 == 0 else nc.gpsimd
                    t = LAW.tile([128, Q], BF, tag="sqw", name=f"nw{ct}")
                    eng.tensor_mul(t[:], xwhat[:, sl], rhow_bt[:, gsl])
                    eng.tensor_sub(xwhat[:, sl], t[:], mrhw_bt[:, gsl])

                dump_tile(1, xwhat[:, 0:512])

            # ========== phase B: projections + BD bounce prefetch =====
            with (
                tc.tile_pool(name="projcw", bufs=1) as PC,
                tc.tile_pool(name="fixw", bufs=2) as FXW,
                tc.tile_pool(name="psP", bufs=2, space="PSUM") as PSP,
                tc.tile_pool(name="psP2", bufs=2, space="PSUM") as PSP2,
            ):
                Wiq_sb = PC.tile([128, 8 * DG], BF, tag="wiq")
                Wq_sb = PC.tile([128, 4 * DG], BF, tag="wq")
                Wr_sb = PC.tile([128, 4 * DG], BF, tag="wr")
                Wk_sb = PC.tile([128, 8 * 512], BF, tag="wk")
                Wv_sb = PC.tile([128, 8 * 512], BF, tag="wv")
                # rel-shift staging (band + VMASK pad), one per parity
                staging = [PC.tile([128, 4 * WBLK], BF, tag=f"stg{i}",
                                   name=f"stg{i}") for i in range(2)]
                for st in staging:
                    nc.gpsimd.memset(st[:], VMASK)
                kbeta_sb = PC.tile([128, 4], F32, tag="kbeta")
                ksum_sb = PC.tile([128, 4], F32, tag="ksum")
                vsum_sb = PC.tile([128, 512], BF, tag="vsum")
                rwb_sb = PC.tile([128, 4], F32, tag="rwb")
                rrb_sb = PC.tile([128, 4], F32, tag="rrb")
                nc.sync.dma_start(kbeta_sb[:], kbeta[:])
                nc.sync.dma_start(ksum_sb[:], ksum[:])
                nc.sync.dma_start(vsum_sb[:], vsum[:])
                nc.sync.dma_start(rwb_sb[:], rwb[:])
                nc.sync.dma_start(rrb_sb[:], rrb[:])
                nc.sync.dma_start(
                    Wiq_sb[:].rearrange("p (kt o) -> p kt o", kt=8),
                    WiqT[:].rearrange("(kt p) o -> p kt o", p=128))
                nc.sync.dma_start(
                    Wq_sb[:].rearrange("p (b o) -> p b o", b=4),
                    WqT[:].rearrange("g (kt p) o -> p (g kt) o", p=128))
                nc.sync.dma_start(
                    Wr_sb[:].rearrange("p (b o) -> p b o", b=4),
                    WrT[:].rearrange("g (kt p) o -> p (g kt) o", p=128))
                rT_sb = SPANWc.tile([128, 4 * KLEN], BF, tag="rtin")
                nc.sync.dma_start(
                    rT_sb[:].rearrange("p (ct j) -> p ct j", ct=4),
                    rT[:].rearrange("(ct p) j -> p ct j", p=128))
                nc.sync.dma_start(
                    Wk_sb[:].rearrange("p (kt o) -> p kt o", kt=8),
                    WkT[:].rearrange("(kt p) o -> p kt o", p=128))
                nc.sync.dma_start(
                    Wv_sb[:].rearrange("p (kt o) -> p kt o", kt=8),
                    WvT[:].rearrange("(kt p) o -> p kt o", p=128))
                nc.sync.dma_start(
                    Wintra_sb[:].rearrange("p (b o) -> p b o", b=4),
                    WintraT[:].rearrange("g (kt p) o -> p (g kt) o", p=128))
                nc.sync.dma_start(
                    Winter_sb[:].rearrange("p (kt o) -> p kt o", kt=4),
                    WinterT[:].rearrange("(kt p) o -> p kt o", p=128))

                # q_global (Wiq over all D): [DG, Q] broadcast into both mts
                ps_qg = PSP2.tile([128, 2 * Q], F32, tag="psqg", bufs=1)
                for mt in range(2):
                    for kt in range(8):
                        nc.tensor.matmul(
                            ps_qg[:, mt * Q:(mt + 1) * Q],
                            Wiq_sb[:, kt * DG + mt * 128:kt * DG + (mt + 1) * 128],
                            xwhat[:, kt * Q:(kt + 1) * Q],
                            start=(kt == 0), stop=(kt == 7))
                qg_sb = SPANWc.tile([128, 2 * Q], BF, tag="qg")
                nc.vector.tensor_copy(qg_sb[:], ps_qg[:])

                for dt in range(4):
                    gl, mt = dt // 2, dt % 2
                    ps_qi = PSP2.tile([128, Q], F32, tag="ps512", name=f"psqi{dt}")
                    for kt in range(2):
                        blk = gl * 2 + kt
                        nc.tensor.matmul(
                            ps_qi[:],
                            Wq_sb[:, blk * DG + mt * 128:blk * DG + (mt + 1) * 128],
                            xwhat[:, blk * Q:(blk + 1) * Q],
                            start=(kt == 0), stop=(kt == 1))
                    nc.vector.scalar_tensor_tensor(
                        QbT[:, dt * Q:(dt + 1) * Q], ps_qi[:], rwb_sb[:, dt:dt + 1],
                        qg_sb[:, mt * Q:(mt + 1) * Q], op0=ALU.add, op1=ALU.add)
                    nc.vector.scalar_tensor_tensor(
                        QcT[:, dt * Q:(dt + 1) * Q], ps_qi[:], rrb_sb[:, dt:dt + 1],
                        qg_sb[:, mt * Q:(mt + 1) * Q], op0=ALU.add, op1=ALU.add)

                for dt in range(4):
                    gl, mt = dt // 2, dt % 2
                    ps_r = PSP.tile([128, KLEN], F32, tag="pskr", name=f"psr{dt}")
                    for kt in range(2):
                        blk = gl * 2 + kt
                        for nb in range(2):
                            nc.tensor.matmul(
                                ps_r[:, nb * 512:(nb + 1) * 512],
                                Wr_sb[:, blk * DG + mt * 128:blk * DG + (mt + 1) * 128],
                                rT_sb[:, blk * KLEN + nb * 512:blk * KLEN + (nb + 1) * 512],
                                start=(kt == 0), stop=(kt == 1))
                    nc.vector.tensor_copy(rhT[:, dt * KLEN:(dt + 1) * KLEN],
                                          ps_r[:])

                # transposed per-token kv stats for the V fixup:
                # rv_sb[:, jt] = rho[jt*128 + p], rv_sb[:, 8+jt] = mrh[...]
                ps_rv = PSP2.tile([128, 512], F32, tag="ps512", name="psrv")
                for jt in range(8):
                    nc.tensor.matmul(ps_rv[:, jt:jt + 1],
                                     rho_bt[:, jt * 128:(jt + 1) * 128],
                                     id_sb[:, 0:1], start=True, stop=True)
                    nc.tensor.matmul(ps_rv[:, 8 + jt:9 + jt],
                                     mrh_bt[:, jt * 128:(jt + 1) * 128],
                                     id_sb[:, 0:1], start=True, stop=True)
                nc.vector.tensor_copy(rv_sb[:], ps_rv[:, 0:16])
                dump_tile(4, QbT[:, 0:512])
                dump_tile(10, QcT[:, 0:512])
                dump_tile(5, rhT[:, 0:1024])

                # ---- BD raw + rel-shift bounce for ALL heads ----
                # even/odd head matmuls adjacent -> PE row-tile packing
                for hp in range(4):
                    dt = hp
                    for it in range(4):
                        jr0 = 384 - it * 128
                        ps_pair = []
                        for par in range(2):
                            off = par * 64
                            qrow = slice(off, off + 64)
                            ps_bd = PSP.tile([128, KLEN], F32, tag="pskr",
                                             name=f"psbd{hp}{par}{it}")
                            for lo, hi in ((jr0, 512), (512, 1024)):
                                nc.tensor.matmul(
                                    ps_bd[:, lo:hi],
                                    QcT[qrow, dt * Q + it * 128:dt * Q + (it + 1) * 128],
                                    rhT[qrow, dt * KLEN + lo:dt * KLEN + hi],
                                    start=True, stop=True)
                            ps_pair.append(ps_bd)
                        for par in range(2):
                            # band copy into staging block [it*WBLK, it*WBLK+J)
                            dst = staging[par][:, it * WBLK:it * WBLK + (1024 - jr0)]
                            if par == 0:
                                nc.vector.tensor_copy(dst, ps_pair[par][:, jr0:KLEN])
                            else:
                                nc.scalar.activation(dst, ps_pair[par][:, jr0:KLEN],
                                                     AF.Copy)
                    for par in range(2):
                        h = 2 * hp + par
                        bdd = bd_dram[h]
                        nc.sync.dma_start(
                            bdd[:].rearrange("p (it j) -> p it j", it=4),
                            staging[par][:].rearrange("p (it j) -> p it j", it=4))
                        for it in range(4):
                            J = 640 + 128 * it
                            shifted = bass.AP(bdd[:].tensor, it * WBLK + 127,
                                              [[4 * WBLK - 1, 128], [1, J]])
                            nc.sync.dma_start(
                                out=scratch[h][:, SOFF[it]:SOFF[it] + J],
                                in_=shifted)

                # ---- K / V projections on RAW kv + LN fixups ----
                for dt in range(4):
                    ps_k = PSP.tile([128, KLEN], F32, tag="pskr", name=f"psk{dt}")
                    for kt in range(8):
                        for nb in range(2):
                            nc.tensor.matmul(
                                ps_k[:, nb * 512:(nb + 1) * 512],
                                Wk_sb[:, kt * 512 + dt * 128:kt * 512 + (dt + 1) * 128],
                                xhat[:, kt * KLEN + nb * 512:kt * KLEN + (nb + 1) * 512],
                                start=(kt == 0), stop=(kt == 7))
                    # K = ps_k*rho + (ksum*mrh + kbeta)  (ksum host-negated);
                    # only the PSUM-reading multiply runs on vector, the
                    # SBUF-only fixup tail goes to gpsimd
                    tk = FXW.tile([128, KLEN], BF, tag="tk", name=f"tk{dt}")
                    tk2 = FXW.tile([128, KLEN], BF, tag="tk2", name=f"tk2{dt}")
                    nc.vector.tensor_mul(tk[:], ps_k[:], rho_bt[:])
                    nc.gpsimd.tensor_scalar(
                        tk2[:], mrh_bt[:], ksum_sb[:, dt:dt + 1],
                        kbeta_sb[:, dt:dt + 1], op0=ALU.mult, op1=ALU.add)
                    nc.gpsimd.tensor_add(
                        KT_sb[:, dt * KLEN:(dt + 1) * KLEN], tk[:], tk2[:])
                for jt in range(8):
                    ps_v = PSP2.tile([128, 512], F32, tag="ps512", name=f"psv{jt}")
                    for kt in range(8):
                        nc.tensor.matmul(
                            ps_v[:],
                            xhat[:, kt * KLEN + jt * 128:kt * KLEN + (jt + 1) * 128],
                            Wv_sb[:, kt * 512:(kt + 1) * 512],
                            start=(kt == 0), stop=(kt == 7))
                    # V^T[t,o] = ps_v*rho_t + vsum_o*mrh_t  (vsum host-negated;
                    # per-token stats come from the transposed rv_sb columns)
                    tv = FXW.tile([128, 512], BF, tag="tv", name=f"tv{jt}")
                    tv2 = FXW.tile([128, 512], BF, tag="tv2", name=f"tv2{jt}")
                    nc.vector.tensor_scalar_mul(tv[:], ps_v[:],
                                                rv_sb[:, jt:jt + 1])
                    nc.gpsimd.tensor_scalar_mul(tv2[:], vsum_sb[:],
                                                rv_sb[:, 8 + jt:9 + jt])
                    vdst = bass.AP(V_sb[:].tensor, V_sb[:].offset + jt * 520,
                                   [[V_sb[:].ap[0][0], 128], [65, 8], [1, 64]])
                    nc.gpsimd.tensor_add(
                        vdst, tv[:].rearrange("p (h c) -> p h c", h=8),
                        tv2[:].rearrange("p (h c) -> p h c", h=8))
                dump_tile(2, KT_sb[:, 0:1024])
                dump_tile(3, V_sb[:, 0:1024])

                # dummy collective: absorb ncfw entry cost well before the
                # real one (gpsimd queue is idle after the V fixups)
                nc.gpsimd.collective_compute(
                    "AllReduce", mybir.AluOpType.add,
                    replica_groups=[[0, 1], [2, 3], [4, 5], [6, 7]],
                    ins=[ccd_in[:]], outs=[ccd_out[:]])
            LNS.__exit__(None, None, None)
            SPANW.__exit__(None, None, None)
            SPAN.__exit__(None, None, None)

            # ====== phase D: attention + fused output transposes ======
            inter_bf = P.tile([128, 2 * 512], BF, tag="interbf")
            with tc.tile_pool(name="psJ", bufs=1, space="PSUM") as PSJ:
                ps_int = [PSJ.tile([128, 512], F32, tag=f"psint{mt}",
                                   name=f"psint{mt}")
                          for mt in range(2)]
                with (
                    tc.tile_pool(name="epool", bufs=3) as EP,
                    tc.tile_pool(name="psS", bufs=3, space="PSUM") as PSS,
                    tc.tile_pool(name="psAV", bufs=2, space="PSUM") as PSAV,
                    tc.tile_pool(name="psT", bufs=1, space="PSUM") as PST,
                ):
                    for hp in range(4):
                        heads = (2 * hp, 2 * hp + 1)
                        dt = hp
                        E_tiles = {h: EP.tile([128, 8 * 512], BF, tag="E",
                                              name=f"E{h}") for h in heads}
                        for jt in range(8):
                            i0 = max(0, jt - 4) * 128
                            for h in heads:
                                par = h % 2
                                off = par * 64
                                qrow = slice(off, off + 64)
                                ps_s = PSS.tile([128, 512], F32, tag="pss",
                                                name=f"pss{h}{jt}")
                                nc.tensor.matmul(
                                    ps_s[:, i0:512],
                                    KT_sb[qrow, dt * KLEN + jt * 128:dt * KLEN + (jt + 1) * 128],
                                    QbT[qrow, dt * Q + i0:dt * Q + 512],
                                    start=True, stop=False, skip_group_check=True)
                                for ib in range(max(0, jt - 4), 4):
                                    nc.tensor.matmul(
                                        ps_s[:, ib * 128:(ib + 1) * 128],
                                        scratch[h][:, SOFF[ib] + jt * 128:
                                                   SOFF[ib] + (jt + 1) * 128],
                                        id_sb[:],
                                        start=False, stop=(ib == 3),
                                        skip_group_check=True)
                                nc.scalar.activation(
                                    E_tiles[h][:, jt * 512 + i0:(jt + 1) * 512],
                                    ps_s[:, i0:512], AF.Exp, scale=SCALE)
                        for it in range(4):
                            for h in heads:
                                ps_av = PSAV.tile([128, 65], F32, tag="psav",
                                                  name=f"psav{h}{it}")
                                jts = VALID[it]
                                for idx, jt in enumerate(jts):
                                    nc.tensor.matmul(
                                        ps_av[:],
                                        E_tiles[h][:, jt * 512 + it * 128:
                                                   jt * 512 + (it + 1) * 128],
                                        V_sb[:, jt * 520 + h * 65:jt * 520 + (h + 1) * 65],
                                        start=(idx == 0), stop=(idx == len(jts) - 1))
                                rec = W.tile([128, 1], F32, tag="rec")
                                nc.vector.reciprocal(rec[:], ps_av[:, 64:65])
                                nc.vector.tensor_scalar_mul(
                                    attn_sb[:, it * 512 + h * 64:it * 512 + (h + 1) * 64],
                                    ps_av[:, 0:64], rec[:])
                        # avT transpose for this head pair's channels (dt)
                        ps_t = PST.tile([128, 512], F32, tag="psavt",
                                        name=f"psavt{dt}")
                        for it in range(4):
                            nc.tensor.matmul(
                                ps_t[:, it * 128:(it + 1) * 128],
                                attn_sb[:, it * 512 + dt * 128:it * 512 + (dt + 1) * 128],
                                id_sb[:], start=True, stop=True)
                        nc.vector.tensor_copy(avT_sb[:, dt * 512:(dt + 1) * 512],
                                              ps_t[:])
                        # inter partial accumulation (kt = dt slice of Winter)
                        for mt in range(2):
                            nc.tensor.matmul(
                                ps_int[mt][:],
                                Winter_sb[:, dt * DG + mt * 128:dt * DG + (mt + 1) * 128],
                                avT_sb[:, dt * 512:(dt + 1) * 512],
                                start=(dt == 0), stop=(dt == 3),
                                skip_group_check=True)
                        if hp == 0:
                            dump_tile(6, E_tiles[0][:, 0:1024])
                            dump_tile(7, E_tiles[0][:, 1536:2560])
                SCR.__exit__(None, None, None)

                # ====== phase E: collective + intra + output ======
                with (
                    tc.tile_pool(name="phE", bufs=1) as PE_,
                    tc.tile_pool(name="psI", bufs=1, space="PSUM") as PSI,
                ):
                    for mt in range(2):
                        nc.vector.tensor_copy(inter_bf[:, mt * 512:(mt + 1) * 512],
                                              ps_int[mt][:])
                        nc.sync.dma_start(cc_in[mt * 128:(mt + 1) * 128, :],
                                          inter_bf[:, mt * 512:(mt + 1) * 512])
                    nc.gpsimd.collective_compute(
                        "AllReduce", mybir.AluOpType.add,
                        replica_groups=[[0, 1], [2, 3], [4, 5], [6, 7]],
                        ins=[cc_in[:]], outs=[cc_out[:]])

                    inter_rd = PE_.tile([128, 2 * 512], BF, tag="interrd")
                    wres_sb = PE_.tile([128, 4 * 512], F32, tag="wres")
                    nc.sync.dma_start(
                        wres_sb[:].rearrange("p (t q) -> p t q", t=4),
                        wres[:].rearrange("(t p) q -> p t q", p=128))
                    intra_ps = []
                    for t in range(4):
                        gl, mt = t // 2, t % 2
                        ps_o = PSI.tile([128, 512], F32, tag=f"psintra{t}")
                        for kt in range(2):
                            blk = gl * 2 + kt
                            nc.tensor.matmul(
                                ps_o[:],
                                Wintra_sb[:, blk * DG + mt * 128:blk * DG + (mt + 1) * 128],
                                avT_sb[:, blk * 512:(blk + 1) * 512],
                                start=(kt == 0), stop=(kt == 1))
                        intra_ps.append(ps_o)

                    for mt in range(2):
                        nc.sync.dma_start(inter_rd[:, mt * 512:(mt + 1) * 512],
                                          cc_out[mt * 128:(mt + 1) * 128, :])

                    out_f = PE_.tile([128, 4 * 512], F32, tag="outf")
                    for t in range(4):
                        mt = t % 2
                        sl = slice(t * 512, (t + 1) * 512)
                        msl = slice(mt * 512, (mt + 1) * 512)
                        tf = W.tile([128, 512], F32, tag="tf")
                        nc.vector.tensor_add(tf[:], intra_ps[t][:], inter_rd[:, msl])
                        nc.vector.tensor_add(out_f[:, sl], tf[:], wres_sb[:, sl])
                        nc.sync.dma_start(out[t * 128:(t + 1) * 128, :], out_f[:, sl])

    nc.finalize()
    return nc


def _host_prep(inputs):
    import concourse.mybir as mybir
    bf = mybir.dt.np(mybir.dt.bfloat16)

    f32 = lambda x: np.ascontiguousarray(np.asarray(x, np.float32))
    tobf = lambda x: np.ascontiguousarray(np.asarray(x, np.float32).astype(bf))

    w = f32(inputs["w"])
    r = f32(inputs["r"])
    mems = f32(inputs["mems"])
    gkv, bkv = f32(inputs["gamma_kv"]), f32(inputs["beta_kv"])
    gq, bq = f32(inputs["gamma_q"]), f32(inputs["beta_q"])
    Wk, Wv = f32(inputs["Wk"]), f32(inputs["Wv"])
    Wq_, Wiq = f32(inputs["Wq"]), f32(inputs["Wiq"])
    Wr_ = f32(inputs["Wr"])
    Wintra, Winter = f32(inputs["Wintra"]), f32(inputs["Winter"])
    rwb_full = f32(inputs["r_w_bias"]).reshape(D)
    rrb_full = f32(inputs["r_r_bias"]).reshape(D)
    kv = np.concatenate([mems, w], 0)
    ident = np.eye(128, dtype=np.float32).astype(bf)

    in_maps = []
    for core in range(8):
        b, s = core // 2, core % 2
        CH0 = 512 * s
        g0, g1 = 2 * s, 2 * s + 1
        perm = np.r_[CH0:CH0 + 512, (512 - CH0):(512 - CH0) + 512]

        qbeta_g = Wiq @ bq
        qbeta = np.concatenate([
            Wq_[g0] @ bq[g0 * DG:(g0 + 1) * DG] + qbeta_g,
            Wq_[g1] @ bq[g1 * DG:(g1 + 1) * DG] + qbeta_g])
        Wkp = Wk[CH0:CH0 + 512, :] * gkv[None, :]
        Wvp = Wv[CH0:CH0 + 512, :] * gkv[None, :]
        m = {
            "kvT": tobf(kv[:, b, :].T),
            "wTp": tobf(w[:, b, perm].T),
            "wres": f32(w[:, b, CH0:CH0 + 512].T),
            "rT": tobf(r[:, 0, CH0:CH0 + 512].T),
            "WkT": tobf(Wkp.T),
            "WvT": tobf(Wvp.T),
            "WiqT": tobf((Wiq * gq[None, :]).T[perm, :]),
            "WqT": tobf(np.stack([
                (Wq_[g] * gq[None, g * DG:(g + 1) * DG]).T for g in (g0, g1)])),
            "WrT": tobf(np.stack([Wr_[g].T for g in (g0, g1)])),
            "WintraT": tobf(np.stack([Wintra[g].T for g in (g0, g1)])),
            "WinterT": tobf(Winter[:, CH0:CH0 + 512].T),
            "kbeta": f32(Wk[CH0:CH0 + 512, :] @ bkv).reshape(4, 128).T,
            "ksum": f32(-Wkp.sum(1)).reshape(4, 128).T,
            "vsum": tobf(np.broadcast_to(-Wvp.sum(1)[None, :], (128, 512))),
            "rwb": f32(rwb_full[CH0:CH0 + 512] + qbeta).reshape(4, 128).T,
            "rrb": f32(rrb_full[CH0:CH0 + 512] + qbeta).reshape(4, 128).T,
            "ident": ident,
        }
        vbeta = Wv[CH0:CH0 + 512, :] @ bkv
        assert np.abs(vbeta).max() < 1e-6, "nonzero beta_kv for V not supported"
        in_maps.append(m)
    return in_maps


def kernel(**inputs):
    from concourse.bass_utils import run_bass_kernel_spmd

    if "nc" not in _cache:
        _cache["nc"] = _build_nc()
    nc = _cache["nc"]
    in_maps = _host_prep(inputs)
    res = run_bass_kernel_spmd(nc, in_maps, core_ids=list(range(8)))
    _cache["last_results"] = res

    full = np.zeros((Q, B, D), np.float32)
    for core in range(8):
        b, s = core // 2, core % 2
        o = np.asarray(res.results[core]["out"], np.float32)   # [512 ch, 512 q]
        full[:, b, 512 * s:512 * s + 512] = o.T
    return full


# revision 53
# speedup vs baseline: 1.1262x; 1.1262x over previous
"""nn_GroupAttention Trainium2 kernel (8-core SPMD), v3.

Sharding: core = (b, s): b = core//2 (batch), s = core%2 (head half).
Each core handles batch b, heads 8s..8s+7 = d_model channels 512s..512s+512
(= groups 2s, 2s+1).  Device tensors are channel-major ("transposed") so all
matmul contractions run over the partition dim.

v3 structure:
- kv LayerNorm is folded into the K/V projections as post-matmul fixups
  (K = (Wk'x)*rho - ksum*mrh + kbeta, V likewise with transposed per-token
  stats), so the K/V matmuls depend only on the raw loads and the serial
  in-place normalize leaves the critical path entirely.
- The transformer-XL rel-shift DRAM bounce for ALL 8 heads is issued right
  after the q/r projections, landing in per-head SBUF scratch tiles; the
  attention inner loop has no DMA dependencies (keeps the PE HAM-warm).
- Scalar-engine activations are batched by function set (squares, then
  ln/exp stats, then copies, then softmax exps) to avoid ACT table reloads.
- avT transposes + inter-projection partial sums run inside the head-pair
  loop; the pair AllReduce (bf16 payload) is issued immediately after the
  last pair and overlaps the intra matmuls.  A dummy early collective
  absorbs the collective firmware entry cost.
"""
import sys

sys.path.insert(0, "/opt/trn_rl_repo")
import numpy as np

Q, M, KLEN, B = 512, 512, 1024, 4
D, H, DH, G = 1024, 16, 64, 4
DG = D // G
EPS = 1e-6
SCALE = 0.125
VMASK = -1e5
WBLK = 1152                 # uniform DRAM/staging block width per query tile
SOFF = [0, 640, 1408, 2304]  # packed scratch offsets (widths 640/768/896/1024)
VALID = {it: [jt for jt in range(8) if jt - it <= 4] for it in range(4)}

_cache = {}


def _build_nc(dump=False):
    import concourse.bass as bass
    import concourse.bacc as bacc
    import concourse.mybir as mybir
    import concourse.tile as tile

    BF = mybir.dt.bfloat16
    F32 = mybir.dt.float32
    AF = mybir.ActivationFunctionType
    ALU = mybir.AluOpType

    nc = bacc.Bacc("TRN2", target_bir_lowering=False, debug=False,
                   num_devices=8)

    def din(name, shape, dt=BF):
        return nc.declare_dram_parameter(name, list(shape), dt, isOutput=False)

    kvT = din("kvT", [D, KLEN])
    wTp = din("wTp", [D, Q])              # channel-PERMUTED w^T
    wres = din("wres", [512, Q], mybir.dt.float32)
    rT = din("rT", [512, KLEN])
    WkT = din("WkT", [D, 512])
    WvT = din("WvT", [D, 512])
    WiqT = din("WiqT", [D, DG])           # row-permuted to match wTp
    WqT = din("WqT", [2, DG, DG])
    WrT = din("WrT", [2, DG, DG])
    WintraT = din("WintraT", [2, DG, DG])
    WinterT = din("WinterT", [512, DG])
    kbeta = din("kbeta", [128, 4], F32)
    ksum = din("ksum", [128, 512])        # row-replicated -Wk' row sums (bf16)
    vsum = din("vsum", [128, 512])        # row-replicated -Wv' row sums (bf16)
    rwb = din("rwb", [128, 4], F32)
    rrb = din("rrb", [128, 4], F32)
    ident = din("ident", [128, 128])
    out = nc.declare_dram_parameter("out", [512, Q], F32, isOutput=True)
    dmp = (nc.declare_dram_parameter("dmp", [128, 12 * 1024], F32, isOutput=True)
           if dump else None)

    def dump_tile(k, ap):
        if dmp is not None:
            nc.gpsimd.dma_start(dmp[:, k * 1024:k * 1024 + ap.shape[-1]], ap)

    bd_dram = [nc.dram_tensor(f"bd_dram{h}", [128, 4 * WBLK], BF)
               for h in range(8)]
    cc_in = nc.dram_tensor("cc_in", [DG, Q], BF)
    cc_out = nc.dram_tensor("cc_out", [DG, Q], BF)
    ccd_in = nc.dram_tensor("ccd_in", [128, 8], BF)
    ccd_out = nc.dram_tensor("ccd_out", [128, 8], BF)

    with tile.TileContext(nc) as tc:
        with (
            tc.tile_pool(name="persist", bufs=1) as P,
            tc.tile_pool(name="work", bufs=2) as W,
        ):
            ones_sq = P.tile([128, 128], BF, tag="onessq")
            nc.gpsimd.memset(ones_sq[:], 1.0)
            id_sb = P.tile([128, 128], BF, tag="id")
            nc.sync.dma_start(id_sb[:], ident[:])

            KT_sb = P.tile([128, 4 * KLEN], BF, tag="KT")       # dt-major
            V_sb = P.tile([128, 8 * 520], BF, tag="V")          # jt x (h x 65)
            QbT = P.tile([128, 4 * Q], BF, tag="QbT")
            QcT = P.tile([128, 4 * Q], BF, tag="QcT")
            rhT = P.tile([128, 4 * KLEN], BF, tag="rhT")        # dt-major
            attn_sb = P.tile([128, 4 * 512], BF, tag="attn")    # it-major
            avT_sb = P.tile([128, 4 * 512], BF, tag="avT")
            Wintra_sb = P.tile([128, 4 * DG], BF, tag="wintra")
            Winter_sb = P.tile([128, 4 * DG], BF, tag="winter")

            # V softmax-denominator ones columns (cols 64 mod 65), static
            for jt in range(8):
                nc.gpsimd.memset(
                    bass.AP(V_sb[:].tensor, V_sb[:].offset + jt * 520 + 64,
                            [[V_sb[:].ap[0][0], 128], [65, 8], [1, 1]]), 1.0)

            # per-head shifted-BD scratch, packed query-tile blocks
            SCR = tc.tile_pool(name="scratch", bufs=1)
            SCRc = SCR.__enter__()
            scratch = [SCRc.tile([128, 3328], BF, tag=f"scr{h}", name=f"scr{h}")
                       for h in range(8)]

            # ================= phase A: stats =================
            SPAN = tc.tile_pool(name="span_kv", bufs=1)
            SPANc = SPAN.__enter__()
            xhat = SPANc.tile([128, 8 * KLEN], BF, tag="xhat")   # RAW kv
            SPANW = tc.tile_pool(name="span_w", bufs=1)
            SPANWc = SPANW.__enter__()
            xwhat = SPANWc.tile([128, 8 * Q], BF, tag="xwhat")
            LNS = tc.tile_pool(name="lnS", bufs=1)
            LNSc = LNS.__enter__()
            rho_bt = LNSc.tile([128, KLEN], BF, tag="rhobt")
            mu_kv = LNSc.tile([128, KLEN], BF, tag="mukv")
            rv_sb = LNSc.tile([128, 16], F32, tag="rvsb")   # rho^T per jt
            with (
                tc.tile_pool(name="lnAa", bufs=1) as LAWa,
                tc.tile_pool(name="lnAw", bufs=2) as LAW,
                tc.tile_pool(name="psA", bufs=1, space="PSUM") as PSA,
            ):
                for ct in range(8):
                    nc.sync.dma_start(
                        xwhat[:, ct * Q:(ct + 1) * Q],
                        wTp[ct * 128:(ct + 1) * 128, :])
                for ct in range(8):
                    nc.sync.dma_start(
                        xhat[:, ct * KLEN:(ct + 1) * KLEN],
                        kvT[ct * 128:(ct + 1) * 128, :])

                # stats in two passes so the scalar engine runs all Ln's then
                # all Exp's (one ACT table switch instead of one per stat)
                def stats_var(ps_sum_ap, ps_sq_ap, n, width, nm, mu_bf=None):
                    mu = LAWa.tile([128, width], F32, tag=f"mu{nm}",
                                   name=f"mu{nm}")
                    t1 = LAWa.tile([128, width], F32, tag=f"t1{nm}",
                                   name=f"t1{nm}")
                    va = LAWa.tile([128, width], F32, tag="va", name=f"va{nm}")
                    nc.vector.tensor_scalar_mul(mu[:], ps_sum_ap, 1.0 / n)
                    nc.vector.tensor_mul(t1[:], mu[:], mu[:])
                    nc.vector.tensor_scalar_mul(va[:], ps_sq_ap, 1.0 / (n - 1))
                    nc.vector.scalar_tensor_tensor(
                        va[:], t1[:], -float(n) / (n - 1), va[:],
                        op0=ALU.mult, op1=ALU.add)
                    if mu_bf is not None:
                        nc.vector.tensor_copy(mu_bf, mu[:])
                    # 1/(sigma+eps) ~= 1/sigma = exp(-0.5*ln(var)); eps err ~1e-6
                    nc.scalar.activation(t1[:], va[:], AF.Ln)
                    return mu, t1

                def stats_rho(mu, t1, rho_out, mrh_out):
                    nc.scalar.activation(rho_out, t1[:], AF.Exp, scale=-0.5)
                    if mrh_out is not None:
                        nc.vector.tensor_mul(mrh_out, mu[:], rho_out)

                rhow_bt = LAWa.tile([128, 4 * Q], BF, tag="rhowbt")
                mrhw_bt = LAWa.tile([128, 4 * Q], BF, tag="mrhwbt")

                # all Square activations first (one ACT table set)
                sqw_tiles = []
                for g in range(4):
                    for kt in range(2):
                        ct = 2 * g + kt
                        sqw = LAW.tile([128, Q], BF, tag="sqw", name=f"sqw{ct}")
                        nc.scalar.activation(sqw[:], xwhat[:, ct * Q:(ct + 1) * Q],
                                             AF.Square)
                        sqw_tiles.append(sqw)
                # w-group stats matmuls
                ps_w = []
                for g in range(4):
                    ps_gs = PSA.tile([128, Q], F32, tag="psg", bufs=4,
                                     name=f"psgs{g}")
                    ps_gq = PSA.tile([128, Q], F32, tag="psg", bufs=4,
                                     name=f"psgq{g}")
                    for kt in range(2):
                        ct = 2 * g + kt
                        nc.tensor.matmul(ps_gs[:], ones_sq[:],
                                         xwhat[:, ct * Q:(ct + 1) * Q],
                                         start=(kt == 0), stop=(kt == 1))
                        nc.tensor.matmul(ps_gq[:], ones_sq[:], sqw_tiles[ct][:],
                                         start=(kt == 0), stop=(kt == 1))
                    ps_w.append((ps_gs, ps_gq))

                # kv stats: squares + all-ones matmuls
                ps_sum = PSA.tile([128, KLEN], F32, tag="pssum")
                ps_sq = PSA.tile([128, KLEN], F32, tag="pssq")
                for ct in range(8):
                    sq = LAW.tile([128, KLEN], BF, tag="sq")
                    nc.scalar.activation(sq[:], xhat[:, ct * KLEN:(ct + 1) * KLEN],
                                         AF.Square)
                    for nb in range(2):
                        sl = slice(nb * 512, (nb + 1) * 512)
                        nc.tensor.matmul(
                            ps_sum[:, sl], ones_sq[:],
                            xhat[:, ct * KLEN + nb * 512:ct * KLEN + (nb + 1) * 512],
                            start=(ct == 0), stop=(ct == 7))
                        nc.tensor.matmul(ps_sq[:, sl], ones_sq[:], sq[:, sl],
                                         start=(ct == 0), stop=(ct == 7))

                # stats: pass 1 (variance + all Ln's), pass 2 (all Exp's)
                st_w = [stats_var(ps_w[g][0][:], ps_w[g][1][:], DG, Q, f"w{g}")
                        for g in range(4)]
                st_kv = stats_var(ps_sum[:], ps_sq[:], D, KLEN, "kv",
                                  mu_bf=mu_kv[:])
                for g in range(4):
                    gsl = slice(g * Q, (g + 1) * Q)
                    stats_rho(st_w[g][0], st_w[g][1],
                              rhow_bt[:, gsl], mrhw_bt[:, gsl])
                stats_rho(st_kv[0], st_kv[1], rho_bt[:], None)

                # normalize xwhat in place (small; q projections consume it)
                for ct in range(8):
                    g = ct // 2
                    sl = slice(ct * Q, (ct + 1) * Q)
                    gsl = slice(g * Q, (g + 1) * Q)
                    eng = nc.vector if ct % 2 == 0 else nc.gpsimd
                    t = LAW.tile([128, Q], BF, tag="sqw", name=f"nw{ct}")
                    eng.tensor_mul(t[:], xwhat[:, sl], rhow_bt[:, gsl])
                    eng.tensor_sub(xwhat[:, sl], t[:], mrhw_bt[:, gsl])

                dump_tile(1, xwhat[:, 0:512])

            # ========== phase B: projections + BD bounce prefetch =====
            with (
                tc.tile_pool(name="projcw", bufs=1) as PC,
                tc.tile_pool(name="fixw", bufs=2) as FXW,
                tc.tile_pool(name="psP", bufs=2, space="PSUM") as PSP,
                tc.tile_pool(name="psP2", bufs=2, space="PSUM") as PSP2,
            ):
                Wiq_sb = PC.tile([128, 8 * DG], BF, tag="wiq")
                Wq_sb = PC.tile([128, 4 * DG], BF, tag="wq")
                Wr_sb = PC.tile([128, 4 * DG], BF, tag="wr")
                Wk_sb = PC.tile([128, 8 * 512], BF, tag="wk")
                Wv_sb = PC.tile([128, 8 * 512], BF, tag="wv")
                # rel-shift staging (band + VMASK pad), one per parity
                staging = [PC.tile([128, 4 * WBLK], BF, tag=f"stg{i}",
                                   name=f"stg{i}") for i in range(2)]
                for st in staging:
                    nc.gpsimd.memset(st[:], VMASK)
                kbeta_sb = PC.tile([128, 4], F32, tag="kbeta")
                ksum_sb = PC.tile([128, 512], BF, tag="ksum")
                vsum_sb = PC.tile([128, 512], BF, tag="vsum")
                rwb_sb = PC.tile([128, 4], F32, tag="rwb")
                rrb_sb = PC.tile([128, 4], F32, tag="rrb")
                nc.sync.dma_start(kbeta_sb[:], kbeta[:])
                nc.sync.dma_start(ksum_sb[:], ksum[:])
                nc.sync.dma_start(vsum_sb[:], vsum[:])
                nc.sync.dma_start(rwb_sb[:], rwb[:])
                nc.sync.dma_start(rrb_sb[:], rrb[:])
                nc.sync.dma_start(
                    Wiq_sb[:].rearrange("p (kt o) -> p kt o", kt=8),
                    WiqT[:].rearrange("(kt p) o -> p kt o", p=128))
                nc.sync.dma_start(
                    Wq_sb[:].rearrange("p (b o) -> p b o", b=4),
                    WqT[:].rearrange("g (kt p) o -> p (g kt) o", p=128))
                nc.sync.dma_start(
                    Wr_sb[:].rearrange("p (b o) -> p b o", b=4),
                    WrT[:].rearrange("g (kt p) o -> p (g kt) o", p=128))
                rT_sb = SPANWc.tile([128, 4 * KLEN], BF, tag="rtin")
                nc.sync.dma_start(
                    rT_sb[:].rearrange("p (ct j) -> p ct j", ct=4),
                    rT[:].rearrange("(ct p) j -> p ct j", p=128))
                nc.sync.dma_start(
                    Wk_sb[:].rearrange("p (kt o) -> p kt o", kt=8),
                    WkT[:].rearrange("(kt p) o -> p kt o", p=128))
                nc.sync.dma_start(
                    Wv_sb[:].rearrange("p (kt o) -> p kt o", kt=8),
                    WvT[:].rearrange("(kt p) o -> p kt o", p=128))
                nc.sync.dma_start(
                    Wintra_sb[:].rearrange("p (b o) -> p b o", b=4),
                    WintraT[:].rearrange("g (kt p) o -> p (g kt) o", p=128))
                nc.sync.dma_start(
                    Winter_sb[:].rearrange("p (kt o) -> p kt o", kt=4),
                    WinterT[:].rearrange("(kt p) o -> p kt o", p=128))

                # ---- K projection on RAW kv (no dependency on normalize);
                # the LN mean-correction accumulates into PSUM as a rank-1
                # outer product: K = (ps - ksum (x) mu) * rho + kbeta
                for dt in range(4):
                    ps_k = PSP.tile([128, KLEN], F32, tag="pskr", name=f"psk{dt}")
                    for kt in range(8):
                        for nb in range(2):
                            nc.tensor.matmul(
                                ps_k[:, nb * 512:(nb + 1) * 512],
                                Wk_sb[:, kt * 512 + dt * 128:kt * 512 + (dt + 1) * 128],
                                xhat[:, kt * KLEN + nb * 512:kt * KLEN + (nb + 1) * 512],
                                start=(kt == 0), stop=False,
                                skip_group_check=True)
                    for nb in range(2):
                        nc.tensor.matmul(
                            ps_k[:, nb * 512:(nb + 1) * 512],
                            ksum_sb[0:1, dt * 128:(dt + 1) * 128],
                            mu_kv[0:1, nb * 512:(nb + 1) * 512],
                            start=False, stop=(nb == 1),
                            skip_group_check=True)
                    tk = FXW.tile([128, KLEN], BF, tag="tk", name=f"tk{dt}")
                    nc.vector.tensor_mul(tk[:], ps_k[:], rho_bt[:])
                    nc.vector.tensor_scalar_add(
                        KT_sb[:, dt * KLEN:(dt + 1) * KLEN], tk[:],
                        kbeta_sb[:, dt:dt + 1])

                # q_global (Wiq over all D): [DG, Q] broadcast into both mts
                ps_qg = PSP2.tile([128, 2 * Q], F32, tag="psqg", bufs=1)
                for mt in range(2):
                    for kt in range(8):
                        nc.tensor.matmul(
                            ps_qg[:, mt * Q:(mt + 1) * Q],
                            Wiq_sb[:, kt * DG + mt * 128:kt * DG + (mt + 1) * 128],
                            xwhat[:, kt * Q:(kt + 1) * Q],
                            start=(kt == 0), stop=(kt == 7))
                qg_sb = SPANWc.tile([128, 2 * Q], BF, tag="qg")
                nc.vector.tensor_copy(qg_sb[:], ps_qg[:])

                for dt in range(4):
                    gl, mt = dt // 2, dt % 2
                    ps_qi = PSP2.tile([128, Q], F32, tag="ps512", name=f"psqi{dt}")
                    for kt in range(2):
                        blk = gl * 2 + kt
                        nc.tensor.matmul(
                            ps_qi[:],
                            Wq_sb[:, blk * DG + mt * 128:blk * DG + (mt + 1) * 128],
                            xwhat[:, blk * Q:(blk + 1) * Q],
                            start=(kt == 0), stop=(kt == 1))
                    nc.vector.scalar_tensor_tensor(
                        QbT[:, dt * Q:(dt + 1) * Q], ps_qi[:], rwb_sb[:, dt:dt + 1],
                        qg_sb[:, mt * Q:(mt + 1) * Q], op0=ALU.add, op1=ALU.add)
                    nc.vector.scalar_tensor_tensor(
                        QcT[:, dt * Q:(dt + 1) * Q], ps_qi[:], rrb_sb[:, dt:dt + 1],
                        qg_sb[:, mt * Q:(mt + 1) * Q], op0=ALU.add, op1=ALU.add)

                for dt in range(4):
                    gl, mt = dt // 2, dt % 2
                    ps_r = PSP.tile([128, KLEN], F32, tag="pskr", name=f"psr{dt}")
                    for kt in range(2):
                        blk = gl * 2 + kt
                        for nb in range(2):
                            nc.tensor.matmul(
                                ps_r[:, nb * 512:(nb + 1) * 512],
                                Wr_sb[:, blk * DG + mt * 128:blk * DG + (mt + 1) * 128],
                                rT_sb[:, blk * KLEN + nb * 512:blk * KLEN + (nb + 1) * 512],
                                start=(kt == 0), stop=(kt == 1))
                    nc.vector.tensor_copy(rhT[:, dt * KLEN:(dt + 1) * KLEN],
                                          ps_r[:])

                # transposed per-token kv 1/sigma for the V fixup:
                # rv_sb[:, jt] = rho[jt*128 + p]
                ps_rv = PSP2.tile([128, 512], F32, tag="ps512", name="psrv")
                for jt in range(8):
                    nc.tensor.matmul(ps_rv[:, jt:jt + 1],
                                     rho_bt[:, jt * 128:(jt + 1) * 128],
                                     id_sb[:, 0:1], start=True, stop=True)
                nc.vector.tensor_copy(rv_sb[:, 0:8], ps_rv[:, 0:8])
                dump_tile(4, QbT[:, 0:512])
                dump_tile(10, QcT[:, 0:512])
                dump_tile(5, rhT[:, 0:1024])

                # ---- BD raw + rel-shift bounce for ALL heads ----
                # even/odd head matmuls adjacent -> PE row-tile packing
                for hp in range(4):
                    dt = hp
                    for it in range(4):
                        jr0 = 384 - it * 128
                        ps_pair = [PSP.tile([128, KLEN], F32, tag="pskr",
                                            name=f"psbd{hp}{par}{it}")
                                   for par in range(2)]
                        # chunk-interleaved so even/odd rows pack on PE tiles
                        for lo, hi in ((jr0, 512), (512, 1024)):
                            for par in range(2):
                                qrow = slice(par * 64, par * 64 + 64)
                                nc.tensor.matmul(
                                    ps_pair[par][:, lo:hi],
                                    QcT[qrow, dt * Q + it * 128:dt * Q + (it + 1) * 128],
                                    rhT[qrow, dt * KLEN + lo:dt * KLEN + hi],
                                    start=True, stop=True)
                        for par in range(2):
                            # band copy into staging block [it*WBLK, it*WBLK+J)
                            dst = staging[par][:, it * WBLK:it * WBLK + (1024 - jr0)]
                            if par == 0:
                                nc.vector.tensor_copy(dst, ps_pair[par][:, jr0:KLEN])
                            else:
                                nc.scalar.activation(dst, ps_pair[par][:, jr0:KLEN],
                                                     AF.Copy)
                    for par in range(2):
                        h = 2 * hp + par
                        bdd = bd_dram[h]
                        nc.sync.dma_start(
                            bdd[:].rearrange("p (it j) -> p it j", it=4),
                            staging[par][:].rearrange("p (it j) -> p it j", it=4))
                        for it in range(4):
                            J = 640 + 128 * it
                            shifted = bass.AP(bdd[:].tensor, it * WBLK + 127,
                                              [[4 * WBLK - 1, 128], [1, J]])
                            nc.sync.dma_start(
                                out=scratch[h][:, SOFF[it]:SOFF[it] + J],
                                in_=shifted)

                # ---- V projection on RAW kv; mean-correction as rank-1
                # accumulation, then V^T = ps_v * rho_t in one strided op
                for jt in range(8):
                    ps_v = PSP2.tile([128, 512], F32, tag="ps512", name=f"psv{jt}")
                    for kt in range(8):
                        nc.tensor.matmul(
                            ps_v[:],
                            xhat[:, kt * KLEN + jt * 128:kt * KLEN + (jt + 1) * 128],
                            Wv_sb[:, kt * 512:(kt + 1) * 512],
                            start=(kt == 0), stop=False,
                            skip_group_check=True)
                    nc.tensor.matmul(
                        ps_v[:], mu_kv[0:1, jt * 128:(jt + 1) * 128],
                        vsum_sb[0:1, :], start=False, stop=True,
                        skip_group_check=True)
                    vdst = bass.AP(V_sb[:].tensor, V_sb[:].offset + jt * 520,
                                   [[V_sb[:].ap[0][0], 128], [65, 8], [1, 64]])
                    nc.vector.tensor_scalar_mul(
                        vdst, ps_v[:].rearrange("p (h c) -> p h c", h=8),
                        rv_sb[:, jt:jt + 1])
                dump_tile(2, KT_sb[:, 0:1024])
                dump_tile(3, V_sb[:, 0:1024])

                # dummy collective: absorb ncfw entry cost well before the
                # real one (gpsimd queue is idle after the V fixups)
                nc.gpsimd.collective_compute(
                    "AllReduce", mybir.AluOpType.add,
                    replica_groups=[[0, 1], [2, 3], [4, 5], [6, 7]],
                    ins=[ccd_in[:]], outs=[ccd_out[:]])
            LNS.__exit__(None, None, None)
            SPANW.__exit__(None, None, None)
            SPAN.__exit__(None, None, None)

            # ====== phase D: attention + fused output transposes ======
            inter_bf = P.tile([128, 2 * 512], BF, tag="interbf")
            with tc.tile_pool(name="psJ", bufs=1, space="PSUM") as PSJ:
                ps_int = [PSJ.tile([128, 512], F32, tag=f"psint{mt}",
                                   name=f"psint{mt}")
                          for mt in range(2)]
                with (
                    tc.tile_pool(name="epool", bufs=3) as EP,
                    tc.tile_pool(name="psS", bufs=3, space="PSUM") as PSS,
                    tc.tile_pool(name="psAV", bufs=2, space="PSUM") as PSAV,
                    tc.tile_pool(name="psT", bufs=1, space="PSUM") as PST,
                ):
                    for hp in range(4):
                        heads = (2 * hp, 2 * hp + 1)
                        dt = hp
                        E_tiles = {h: EP.tile([128, 8 * 512], BF, tag="E",
                                              name=f"E{h}") for h in heads}
                        for jt in range(8):
                            i0 = max(0, jt - 4) * 128
                            # both heads' K=64 AC matmuls adjacent -> they
                            # pack onto independent PE row-tiles
                            ps_j = {}
                            for h in heads:
                                qrow = slice((h % 2) * 64, (h % 2) * 64 + 64)
                                ps_s = PSS.tile([128, 512], F32, tag="pss",
                                                name=f"pss{h}{jt}")
                                ps_j[h] = ps_s
                                nc.tensor.matmul(
                                    ps_s[:, i0:512],
                                    KT_sb[qrow, dt * KLEN + jt * 128:dt * KLEN + (jt + 1) * 128],
                                    QbT[qrow, dt * Q + i0:dt * Q + 512],
                                    start=True, stop=False, skip_group_check=True)
                            for h in heads:
                                for ib in range(max(0, jt - 4), 4):
                                    nc.tensor.matmul(
                                        ps_j[h][:, ib * 128:(ib + 1) * 128],
                                        scratch[h][:, SOFF[ib] + jt * 128:
                                                   SOFF[ib] + (jt + 1) * 128],
                                        id_sb[:],
                                        start=False, stop=(ib == 3),
                                        skip_group_check=True)
                                nc.scalar.activation(
                                    E_tiles[h][:, jt * 512 + i0:(jt + 1) * 512],
                                    ps_j[h][:, i0:512], AF.Exp, scale=SCALE)
                        for it in range(4):
                            for h in heads:
                                ps_av = PSAV.tile([128, 65], F32, tag="psav",
                                                  name=f"psav{h}{it}")
                                jts = VALID[it]
                                for idx, jt in enumerate(jts):
                                    nc.tensor.matmul(
                                        ps_av[:],
                                        E_tiles[h][:, jt * 512 + it * 128:
                                                   jt * 512 + (it + 1) * 128],
                                        V_sb[:, jt * 520 + h * 65:jt * 520 + (h + 1) * 65],
                                        start=(idx == 0), stop=(idx == len(jts) - 1))
                                rec = W.tile([128, 1], F32, tag="rec")
                                nc.vector.reciprocal(rec[:], ps_av[:, 64:65])
                                nc.vector.tensor_scalar_mul(
                                    attn_sb[:, it * 512 + h * 64:it * 512 + (h + 1) * 64],
                                    ps_av[:, 0:64], rec[:])
                        # avT transpose for this head pair's channels (dt)
                        ps_t = PST.tile([128, 512], F32, tag="psavt",
                                        name=f"psavt{dt}")
                        for it in range(4):
                            nc.tensor.matmul(
                                ps_t[:, it * 128:(it + 1) * 128],
                                attn_sb[:, it * 512 + dt * 128:it * 512 + (dt + 1) * 128],
                                id_sb[:], start=True, stop=True)
                        nc.vector.tensor_copy(avT_sb[:, dt * 512:(dt + 1) * 512],
                                              ps_t[:])
                        # inter partial accumulation (kt = dt slice of Winter)
                        for mt in range(2):
                            nc.tensor.matmul(
                                ps_int[mt][:],
                                Winter_sb[:, dt * DG + mt * 128:dt * DG + (mt + 1) * 128],
                                avT_sb[:, dt * 512:(dt + 1) * 512],
                                start=(dt == 0), stop=(dt == 3),
                                skip_group_check=True)
                        if hp == 0:
                            dump_tile(6, E_tiles[0][:, 0:1024])
                            dump_tile(7, E_tiles[0][:, 1536:2560])
                SCR.__exit__(None, None, None)

                # ====== phase E: collective + intra + output ======
                with (
                    tc.tile_pool(name="phE", bufs=1) as PE_,
                    tc.tile_pool(name="psI", bufs=1, space="PSUM") as PSI,
                ):
                    for mt in range(2):
                        nc.vector.tensor_copy(inter_bf[:, mt * 512:(mt + 1) * 512],
                                              ps_int[mt][:])
                        nc.sync.dma_start(cc_in[mt * 128:(mt + 1) * 128, :],
                                          inter_bf[:, mt * 512:(mt + 1) * 512])
                    nc.gpsimd.collective_compute(
                        "AllReduce", mybir.AluOpType.add,
                        replica_groups=[[0, 1], [2, 3], [4, 5], [6, 7]],
                        ins=[cc_in[:]], outs=[cc_out[:]])

                    inter_rd = PE_.tile([128, 2 * 512], BF, tag="interrd")
                    wres_sb = PE_.tile([128, 4 * 512], F32, tag="wres")
                    nc.sync.dma_start(
                        wres_sb[:].rearrange("p (t q) -> p t q", t=4),
                        wres[:].rearrange("(t p) q -> p t q", p=128))
                    intra_ps = []
                    for t in range(4):
                        gl, mt = t // 2, t % 2
                        ps_o = PSI.tile([128, 512], F32, tag=f"psintra{t}")
                        for kt in range(2):
                            blk = gl * 2 + kt
                            nc.tensor.matmul(
                                ps_o[:],
                                Wintra_sb[:, blk * DG + mt * 128:blk * DG + (mt + 1) * 128],
                                avT_sb[:, blk * 512:(blk + 1) * 512],
                                start=(kt == 0), stop=(kt == 1))
                        intra_ps.append(ps_o)

                    for mt in range(2):
                        nc.sync.dma_start(inter_rd[:, mt * 512:(mt + 1) * 512],
                                          cc_out[mt * 128:(mt + 1) * 128, :])

                    out_f = PE_.tile([128, 4 * 512], F32, tag="outf")
                    for t in range(4):
                        mt = t % 2
                        sl = slice(t * 512, (t + 1) * 512)
                        msl = slice(mt * 512, (mt + 1) * 512)
                        tf = W.tile([128, 512], F32, tag="tf")
                        nc.vector.tensor_add(tf[:], intra_ps[t][:], inter_rd[:, msl])
                        nc.vector.tensor_add(out_f[:, sl], tf[:], wres_sb[:, sl])
                        nc.sync.dma_start(out[t * 128:(t + 1) * 128, :], out_f[:, sl])

    nc.finalize()
    return nc


def _host_prep(inputs):
    import concourse.mybir as mybir
    bf = mybir.dt.np(mybir.dt.bfloat16)

    f32 = lambda x: np.ascontiguousarray(np.asarray(x, np.float32))
    tobf = lambda x: np.ascontiguousarray(np.asarray(x, np.float32).astype(bf))

    w = f32(inputs["w"])
    r = f32(inputs["r"])
    mems = f32(inputs["mems"])
    gkv, bkv = f32(inputs["gamma_kv"]), f32(inputs["beta_kv"])
    gq, bq = f32(inputs["gamma_q"]), f32(inputs["beta_q"])
    Wk, Wv = f32(inputs["Wk"]), f32(inputs["Wv"])
    Wq_, Wiq = f32(inputs["Wq"]), f32(inputs["Wiq"])
    Wr_ = f32(inputs["Wr"])
    Wintra, Winter = f32(inputs["Wintra"]), f32(inputs["Winter"])
    rwb_full = f32(inputs["r_w_bias"]).reshape(D)
    rrb_full = f32(inputs["r_r_bias"]).reshape(D)
    kv = np.concatenate([mems, w], 0)
    ident = np.eye(128, dtype=np.float32).astype(bf)

    in_maps = []
    for core in range(8):
        b, s = core // 2, core % 2
        CH0 = 512 * s
        g0, g1 = 2 * s, 2 * s + 1
        perm = np.r_[CH0:CH0 + 512, (512 - CH0):(512 - CH0) + 512]

        qbeta_g = Wiq @ bq
        qbeta = np.concatenate([
            Wq_[g0] @ bq[g0 * DG:(g0 + 1) * DG] + qbeta_g,
            Wq_[g1] @ bq[g1 * DG:(g1 + 1) * DG] + qbeta_g])
        Wkp = Wk[CH0:CH0 + 512, :] * gkv[None, :]
        Wvp = Wv[CH0:CH0 + 512, :] * gkv[None, :]
        m = {
            "kvT": tobf(kv[:, b, :].T),
            "wTp": tobf(w[:, b, perm].T),
            "wres": f32(w[:, b, CH0:CH0 + 512].T),
            "rT": tobf(r[:, 0, CH0:CH0 + 512].T),
            "WkT": tobf(Wkp.T),
            "WvT": tobf(Wvp.T),
            "WiqT": tobf((Wiq * gq[None, :]).T[perm, :]),
            "WqT": tobf(np.stack([
                (Wq_[g] * gq[None, g * DG:(g + 1) * DG]).T for g in (g0, g1)])),
            "WrT": tobf(np.stack([Wr_[g].T for g in (g0, g1)])),
            "WintraT": tobf(np.stack([Wintra[g].T for g in (g0, g1)])),
            "WinterT": tobf(Winter[:, CH0:CH0 + 512].T),
            "kbeta": f32(Wk[CH0:CH0 + 512, :] @ bkv).reshape(4, 128).T,
            "ksum": tobf(np.broadcast_to(-Wkp.sum(1)[None, :], (128, 512))),
            "vsum": tobf(np.broadcast_to(-Wvp.sum(1)[None, :], (128, 512))),
            "rwb": f32(rwb_full[CH0:CH0 + 512] + qbeta).reshape(4, 128).T,
            "rrb": f32(rrb_full[CH0:CH0 + 512] + qbeta).reshape(4, 128).T,
            "ident": ident,
        }
        vbeta = Wv[CH0:CH0 + 512, :] @ bkv
        assert np.abs(vbeta).max() < 1e-6, "nonzero beta_kv for V not supported"
        in_maps.append(m)
    return in_maps


def kernel(**inputs):
    from concourse.bass_utils import run_bass_kernel_spmd

    if "nc" not in _cache:
        _cache["nc"] = _build_nc()
    nc = _cache["nc"]
    in_maps = _host_prep(inputs)
    res = run_bass_kernel_spmd(nc, in_maps, core_ids=list(range(8)))
    _cache["last_results"] = res

    full = np.zeros((Q, B, D), np.float32)
    for core in range(8):
        b, s = core // 2, core % 2
        o = np.asarray(res.results[core]["out"], np.float32)   # [512 ch, 512 q]
        full[:, b, 512 * s:512 * s + 512] = o.T
    return full


# revision 56
# speedup vs baseline: 1.1657x; 1.0351x over previous
"""nn_GroupAttention Trainium2 kernel (8-core SPMD), v3.

Sharding: core = (b, s): b = core//2 (batch), s = core%2 (head half).
Each core handles batch b, heads 8s..8s+7 = d_model channels 512s..512s+512
(= groups 2s, 2s+1).  Device tensors are channel-major ("transposed") so all
matmul contractions run over the partition dim.

v3 structure:
- kv LayerNorm is folded into the K/V projections as post-matmul fixups
  (K = (Wk'x)*rho - ksum*mrh + kbeta, V likewise with transposed per-token
  stats), so the K/V matmuls depend only on the raw loads and the serial
  in-place normalize leaves the critical path entirely.
- The transformer-XL rel-shift DRAM bounce for ALL 8 heads is issued right
  after the q/r projections, landing in per-head SBUF scratch tiles; the
  attention inner loop has no DMA dependencies (keeps the PE HAM-warm).
- Scalar-engine activations are batched by function set (squares, then
  ln/exp stats, then copies, then softmax exps) to avoid ACT table reloads.
- avT transposes + inter-projection partial sums run inside the head-pair
  loop; the pair AllReduce (bf16 payload) is issued immediately after the
  last pair and overlaps the intra matmuls.  A dummy early collective
  absorbs the collective firmware entry cost.
"""
import sys

sys.path.insert(0, "/opt/trn_rl_repo")
import numpy as np

Q, M, KLEN, B = 512, 512, 1024, 4
D, H, DH, G = 1024, 16, 64, 4
DG = D // G
EPS = 1e-6
SCALE = 0.125
VMASK = -1e5
WBLK = 1152                 # uniform DRAM/staging block width per query tile
SOFF = [0, 640, 1408, 2304]  # packed scratch offsets (widths 640/768/896/1024)
VALID = {it: [jt for jt in range(8) if jt - it <= 4] for it in range(4)}

_cache = {}


def _build_nc(dump=False):
    import concourse.bass as bass
    import concourse.bacc as bacc
    import concourse.mybir as mybir
    import concourse.tile as tile

    BF = mybir.dt.bfloat16
    F32 = mybir.dt.float32
    AF = mybir.ActivationFunctionType
    ALU = mybir.AluOpType

    nc = bacc.Bacc("TRN2", target_bir_lowering=False, debug=False,
                   num_devices=8)

    def din(name, shape, dt=BF):
        return nc.declare_dram_parameter(name, list(shape), dt, isOutput=False)

    kvT = din("kvT", [D, KLEN])
    wTp = din("wTp", [D, Q])              # channel-PERMUTED w^T
    wres = din("wres", [512, Q], mybir.dt.float32)
    rT = din("rT", [512, KLEN])
    WkT = din("WkT", [D, 512])
    WvT = din("WvT", [D, 512])
    WiqT = din("WiqT", [D, DG])           # row-permuted to match wTp
    WqT = din("WqT", [2, DG, DG])
    WrT = din("WrT", [2, DG, DG])
    WintraT = din("WintraT", [2, DG, DG])
    WinterT = din("WinterT", [512, DG])
    kbeta = din("kbeta", [128, 4], F32)
    ksum = din("ksum", [128, 512])        # row-replicated -Wk' row sums (bf16)
    vsum = din("vsum", [128, 512])        # row-replicated -Wv' row sums (bf16)
    rwb = din("rwb", [128, 4], F32)
    rrb = din("rrb", [128, 4], F32)
    ident = din("ident", [128, 128])
    out = nc.declare_dram_parameter("out", [512, Q], F32, isOutput=True)
    dmp = (nc.declare_dram_parameter("dmp", [128, 12 * 1024], F32, isOutput=True)
           if dump else None)

    def dump_tile(k, ap):
        if dmp is not None:
            nc.gpsimd.dma_start(dmp[:, k * 1024:k * 1024 + ap.shape[-1]], ap)

    bd_dram = [nc.dram_tensor(f"bd_dram{h}", [128, 4 * WBLK], BF)
               for h in range(8)]
    cc_in = nc.dram_tensor("cc_in", [DG, Q], BF)
    cc_out = nc.dram_tensor("cc_out", [DG, Q], BF)
    ccd_in = nc.dram_tensor("ccd_in", [128, 8], BF)
    ccd_out = nc.dram_tensor("ccd_out", [128, 8], BF)

    with tile.TileContext(nc) as tc:
        with (
            tc.tile_pool(name="persist", bufs=1) as P,
            tc.tile_pool(name="work", bufs=2) as W,
        ):
            ones_sq = P.tile([128, 128], BF, tag="onessq")
            nc.gpsimd.memset(ones_sq[:], 1.0)
            id_sb = P.tile([128, 128], BF, tag="id")
            nc.sync.dma_start(id_sb[:], ident[:])

            KT_sb = P.tile([128, 4 * KLEN], BF, tag="KT")       # dt-major
            V_sb = P.tile([128, 8 * 520], BF, tag="V")          # jt x (h x 65)
            QbT = P.tile([128, 4 * Q], BF, tag="QbT")
            QcT = P.tile([128, 4 * Q], BF, tag="QcT")
            rhT = P.tile([128, 4 * KLEN], BF, tag="rhT")        # dt-major
            attn_sb = P.tile([128, 4 * 512], BF, tag="attn")    # it-major
            avT_sb = P.tile([128, 4 * 512], BF, tag="avT")
            Wintra_sb = P.tile([128, 4 * DG], BF, tag="wintra")
            Winter_sb = P.tile([128, 4 * DG], BF, tag="winter")

            # V softmax-denominator ones columns (cols 64 mod 65), static
            for jt in range(8):
                nc.gpsimd.memset(
                    bass.AP(V_sb[:].tensor, V_sb[:].offset + jt * 520 + 64,
                            [[V_sb[:].ap[0][0], 128], [65, 8], [1, 1]]), 1.0)

            # per-head shifted-BD scratch, packed query-tile blocks
            SCR = tc.tile_pool(name="scratch", bufs=1)
            SCRc = SCR.__enter__()
            scratch = [SCRc.tile([128, 3328], BF, tag=f"scr{h}", name=f"scr{h}")
                       for h in range(8)]

            # ================= phase A: stats =================
            SPAN = tc.tile_pool(name="span_kv", bufs=1)
            SPANc = SPAN.__enter__()
            xhat = SPANc.tile([128, 8 * KLEN], BF, tag="xhat")   # RAW kv
            SPANW = tc.tile_pool(name="span_w", bufs=1)
            SPANWc = SPANW.__enter__()
            xwhat = SPANWc.tile([128, 8 * Q], BF, tag="xwhat")
            LNS = tc.tile_pool(name="lnS", bufs=1)
            LNSc = LNS.__enter__()
            rho_bt = LNSc.tile([128, KLEN], BF, tag="rhobt")
            mu_kv = LNSc.tile([128, KLEN], BF, tag="mukv")
            rv_sb = LNSc.tile([128, 16], F32, tag="rvsb")   # rho^T per jt
            with (
                tc.tile_pool(name="lnAa", bufs=1) as LAWa,
                tc.tile_pool(name="lnAw", bufs=2) as LAW,
                tc.tile_pool(name="psA", bufs=1, space="PSUM") as PSA,
            ):
                for ct in range(8):
                    nc.sync.dma_start(
                        xwhat[:, ct * Q:(ct + 1) * Q],
                        wTp[ct * 128:(ct + 1) * 128, :])
                for ct in range(8):
                    nc.sync.dma_start(
                        xhat[:, ct * KLEN:(ct + 1) * KLEN],
                        kvT[ct * 128:(ct + 1) * 128, :])

                # stats in two passes so the scalar engine runs all Ln's then
                # all Exp's (one ACT table switch instead of one per stat)
                def stats_var(ps_sum_ap, ps_sq_ap, n, width, nm, mu_bf=None):
                    mu = LAWa.tile([128, width], F32, tag=f"mu{nm}",
                                   name=f"mu{nm}")
                    t1 = LAWa.tile([128, width], F32, tag=f"t1{nm}",
                                   name=f"t1{nm}")
                    va = LAWa.tile([128, width], F32, tag="va", name=f"va{nm}")
                    nc.vector.tensor_scalar_mul(mu[:], ps_sum_ap, 1.0 / n)
                    nc.vector.tensor_mul(t1[:], mu[:], mu[:])
                    nc.vector.tensor_scalar_mul(va[:], ps_sq_ap, 1.0 / (n - 1))
                    nc.vector.scalar_tensor_tensor(
                        va[:], t1[:], -float(n) / (n - 1), va[:],
                        op0=ALU.mult, op1=ALU.add)
                    if mu_bf is not None:
                        nc.vector.tensor_copy(mu_bf, mu[:])
                    # 1/(sigma+eps) ~= 1/sigma = exp(-0.5*ln(var)); eps err ~1e-6
                    nc.scalar.activation(t1[:], va[:], AF.Ln)
                    return mu, t1

                def stats_rho(mu, t1, rho_out, mrh_out):
                    nc.scalar.activation(rho_out, t1[:], AF.Exp, scale=-0.5)
                    if mrh_out is not None:
                        nc.vector.tensor_mul(mrh_out, mu[:], rho_out)

                rhow_bt = LAWa.tile([128, 4 * Q], BF, tag="rhowbt")
                mrhw_bt = LAWa.tile([128, 4 * Q], BF, tag="mrhwbt")

                # all Square activations first (one ACT table set)
                sqw_tiles = []
                for g in range(4):
                    for kt in range(2):
                        ct = 2 * g + kt
                        sqw = LAW.tile([128, Q], BF, tag="sqw", name=f"sqw{ct}")
                        nc.scalar.activation(sqw[:], xwhat[:, ct * Q:(ct + 1) * Q],
                                             AF.Square)
                        sqw_tiles.append(sqw)
                # w-group stats matmuls
                ps_w = []
                for g in range(4):
                    ps_gs = PSA.tile([128, Q], F32, tag="psg", bufs=4,
                                     name=f"psgs{g}")
                    ps_gq = PSA.tile([128, Q], F32, tag="psg", bufs=4,
                                     name=f"psgq{g}")
                    for kt in range(2):
                        ct = 2 * g + kt
                        nc.tensor.matmul(ps_gs[:], ones_sq[:],
                                         xwhat[:, ct * Q:(ct + 1) * Q],
                                         start=(kt == 0), stop=(kt == 1))
                        nc.tensor.matmul(ps_gq[:], ones_sq[:], sqw_tiles[ct][:],
                                         start=(kt == 0), stop=(kt == 1))
                    ps_w.append((ps_gs, ps_gq))

                # kv stats: squares + all-ones matmuls
                ps_sum = PSA.tile([128, KLEN], F32, tag="pssum")
                ps_sq = PSA.tile([128, KLEN], F32, tag="pssq")
                for ct in range(8):
                    sq = LAW.tile([128, KLEN], BF, tag="sq")
                    nc.scalar.activation(sq[:], xhat[:, ct * KLEN:(ct + 1) * KLEN],
                                         AF.Square)
                    for nb in range(2):
                        sl = slice(nb * 512, (nb + 1) * 512)
                        nc.tensor.matmul(
                            ps_sum[:, sl], ones_sq[:],
                            xhat[:, ct * KLEN + nb * 512:ct * KLEN + (nb + 1) * 512],
                            start=(ct == 0), stop=(ct == 7))
                        nc.tensor.matmul(ps_sq[:, sl], ones_sq[:], sq[:, sl],
                                         start=(ct == 0), stop=(ct == 7))

                # stats: pass 1 (variance + all Ln's), pass 2 (all Exp's)
                st_w = [stats_var(ps_w[g][0][:], ps_w[g][1][:], DG, Q, f"w{g}")
                        for g in range(4)]
                st_kv = stats_var(ps_sum[:], ps_sq[:], D, KLEN, "kv",
                                  mu_bf=mu_kv[:])
                for g in range(4):
                    gsl = slice(g * Q, (g + 1) * Q)
                    stats_rho(st_w[g][0], st_w[g][1],
                              rhow_bt[:, gsl], mrhw_bt[:, gsl])
                stats_rho(st_kv[0], st_kv[1], rho_bt[:], None)

                # normalize xwhat in place (small; q projections consume it)
                for ct in range(8):
                    g = ct // 2
                    sl = slice(ct * Q, (ct + 1) * Q)
                    gsl = slice(g * Q, (g + 1) * Q)
                    eng = nc.vector if ct % 2 == 0 else nc.gpsimd
                    t = LAW.tile([128, Q], BF, tag="sqw", name=f"nw{ct}")
                    eng.tensor_mul(t[:], xwhat[:, sl], rhow_bt[:, gsl])
                    eng.tensor_sub(xwhat[:, sl], t[:], mrhw_bt[:, gsl])

                dump_tile(1, xwhat[:, 0:512])

            # ========== phase B: projections + BD bounce prefetch =====
            with (
                tc.tile_pool(name="projcw", bufs=1) as PC,
                tc.tile_pool(name="fixw", bufs=2) as FXW,
                tc.tile_pool(name="psP", bufs=2, space="PSUM") as PSP,
                tc.tile_pool(name="psP2", bufs=2, space="PSUM") as PSP2,
            ):
                Wiq_sb = PC.tile([128, 8 * DG], BF, tag="wiq")
                Wq_sb = PC.tile([128, 4 * DG], BF, tag="wq")
                Wr_sb = PC.tile([128, 4 * DG], BF, tag="wr")
                Wk_sb = PC.tile([128, 8 * 512], BF, tag="wk")
                Wv_sb = PC.tile([128, 8 * 512], BF, tag="wv")
                # rel-shift staging (band + VMASK pad), one per parity
                staging = [PC.tile([128, 4 * WBLK], BF, tag=f"stg{i}",
                                   name=f"stg{i}") for i in range(2)]
                for st in staging:
                    nc.gpsimd.memset(st[:], VMASK)
                kbeta_sb = PC.tile([128, 4], F32, tag="kbeta")
                ksum_sb = PC.tile([128, 512], BF, tag="ksum")
                vsum_sb = PC.tile([128, 512], BF, tag="vsum")
                rwb_sb = PC.tile([128, 4], F32, tag="rwb")
                rrb_sb = PC.tile([128, 4], F32, tag="rrb")
                nc.sync.dma_start(kbeta_sb[:], kbeta[:])
                nc.sync.dma_start(ksum_sb[:], ksum[:])
                nc.sync.dma_start(vsum_sb[:], vsum[:])
                nc.sync.dma_start(rwb_sb[:], rwb[:])
                nc.sync.dma_start(rrb_sb[:], rrb[:])
                nc.sync.dma_start(
                    Wiq_sb[:].rearrange("p (kt o) -> p kt o", kt=8),
                    WiqT[:].rearrange("(kt p) o -> p kt o", p=128))
                nc.sync.dma_start(
                    Wq_sb[:].rearrange("p (b o) -> p b o", b=4),
                    WqT[:].rearrange("g (kt p) o -> p (g kt) o", p=128))
                nc.sync.dma_start(
                    Wr_sb[:].rearrange("p (b o) -> p b o", b=4),
                    WrT[:].rearrange("g (kt p) o -> p (g kt) o", p=128))
                rT_sb = SPANWc.tile([128, 4 * KLEN], BF, tag="rtin")
                nc.sync.dma_start(
                    rT_sb[:].rearrange("p (ct j) -> p ct j", ct=4),
                    rT[:].rearrange("(ct p) j -> p ct j", p=128))
                nc.sync.dma_start(
                    Wk_sb[:].rearrange("p (kt o) -> p kt o", kt=8),
                    WkT[:].rearrange("(kt p) o -> p kt o", p=128))
                nc.sync.dma_start(
                    Wv_sb[:].rearrange("p (kt o) -> p kt o", kt=8),
                    WvT[:].rearrange("(kt p) o -> p kt o", p=128))
                nc.sync.dma_start(
                    Wintra_sb[:].rearrange("p (b o) -> p b o", b=4),
                    WintraT[:].rearrange("g (kt p) o -> p (g kt) o", p=128))
                nc.sync.dma_start(
                    Winter_sb[:].rearrange("p (kt o) -> p kt o", kt=4),
                    WinterT[:].rearrange("(kt p) o -> p kt o", p=128))

                # ---- K projection on RAW kv (no dependency on normalize);
                # the LN mean-correction accumulates into PSUM as a rank-1
                # outer product: K = (ps - ksum (x) mu) * rho + kbeta
                for dt in range(4):
                    ps_k = PSP.tile([128, KLEN], F32, tag="pskr", name=f"psk{dt}")
                    for kt in range(8):
                        for nb in range(2):
                            nc.tensor.matmul(
                                ps_k[:, nb * 512:(nb + 1) * 512],
                                Wk_sb[:, kt * 512 + dt * 128:kt * 512 + (dt + 1) * 128],
                                xhat[:, kt * KLEN + nb * 512:kt * KLEN + (nb + 1) * 512],
                                start=(kt == 0), stop=False,
                                skip_group_check=True)
                    for nb in range(2):
                        nc.tensor.matmul(
                            ps_k[:, nb * 512:(nb + 1) * 512],
                            ksum_sb[0:1, dt * 128:(dt + 1) * 128],
                            mu_kv[0:1, nb * 512:(nb + 1) * 512],
                            start=False, stop=(nb == 1),
                            skip_group_check=True)
                    tk = FXW.tile([128, KLEN], BF, tag="tk", name=f"tk{dt}")
                    nc.vector.tensor_mul(tk[:], ps_k[:], rho_bt[:])
                    nc.vector.tensor_scalar_add(
                        KT_sb[:, dt * KLEN:(dt + 1) * KLEN], tk[:],
                        kbeta_sb[:, dt:dt + 1])

                # q_global (Wiq over all D): [DG, Q] broadcast into both mts
                ps_qg = PSP2.tile([128, 2 * Q], F32, tag="psqg", bufs=1)
                for mt in range(2):
                    for kt in range(8):
                        nc.tensor.matmul(
                            ps_qg[:, mt * Q:(mt + 1) * Q],
                            Wiq_sb[:, kt * DG + mt * 128:kt * DG + (mt + 1) * 128],
                            xwhat[:, kt * Q:(kt + 1) * Q],
                            start=(kt == 0), stop=(kt == 7))
                qg_sb = SPANWc.tile([128, 2 * Q], BF, tag="qg")
                nc.vector.tensor_copy(qg_sb[:], ps_qg[:])

                for dt in range(4):
                    gl, mt = dt // 2, dt % 2
                    ps_qi = PSP2.tile([128, Q], F32, tag="ps512", name=f"psqi{dt}")
                    for kt in range(2):
                        blk = gl * 2 + kt
                        nc.tensor.matmul(
                            ps_qi[:],
                            Wq_sb[:, blk * DG + mt * 128:blk * DG + (mt + 1) * 128],
                            xwhat[:, blk * Q:(blk + 1) * Q],
                            start=(kt == 0), stop=(kt == 1))
                    nc.vector.scalar_tensor_tensor(
                        QbT[:, dt * Q:(dt + 1) * Q], ps_qi[:], rwb_sb[:, dt:dt + 1],
                        qg_sb[:, mt * Q:(mt + 1) * Q], op0=ALU.add, op1=ALU.add)
                    nc.vector.scalar_tensor_tensor(
                        QcT[:, dt * Q:(dt + 1) * Q], ps_qi[:], rrb_sb[:, dt:dt + 1],
                        qg_sb[:, mt * Q:(mt + 1) * Q], op0=ALU.add, op1=ALU.add)

                for dt in range(4):
                    gl, mt = dt // 2, dt % 2
                    ps_r = PSP.tile([128, KLEN], F32, tag="pskr", name=f"psr{dt}")
                    for kt in range(2):
                        blk = gl * 2 + kt
                        for nb in range(2):
                            nc.tensor.matmul(
                                ps_r[:, nb * 512:(nb + 1) * 512],
                                Wr_sb[:, blk * DG + mt * 128:blk * DG + (mt + 1) * 128],
                                rT_sb[:, blk * KLEN + nb * 512:blk * KLEN + (nb + 1) * 512],
                                start=(kt == 0), stop=(kt == 1))
                    nc.vector.tensor_copy(rhT[:, dt * KLEN:(dt + 1) * KLEN],
                                          ps_r[:])

                # transposed per-token kv 1/sigma for the V fixup:
                # rv_sb[:, jt] = rho[jt*128 + p]
                ps_rv = PSP2.tile([128, 512], F32, tag="ps512", name="psrv")
                for jt in range(8):
                    nc.tensor.matmul(ps_rv[:, jt:jt + 1],
                                     rho_bt[:, jt * 128:(jt + 1) * 128],
                                     id_sb[:, 0:1], start=True, stop=True)
                nc.vector.tensor_copy(rv_sb[:, 0:8], ps_rv[:, 0:8])
                dump_tile(4, QbT[:, 0:512])
                dump_tile(10, QcT[:, 0:512])
                dump_tile(5, rhT[:, 0:1024])

                # ---- BD raw + rel-shift bounce for ALL heads ----
                # even/odd head matmuls adjacent -> PE row-tile packing
                for hp in range(4):
                    dt = hp
                    for it in range(4):
                        jr0 = 384 - it * 128
                        ps_pair = [PSP.tile([128, KLEN], F32, tag="pskr",
                                            name=f"psbd{hp}{par}{it}")
                                   for par in range(2)]
                        # chunk-interleaved so even/odd rows pack on PE tiles
                        for lo, hi in ((jr0, 512), (512, 1024)):
                            for par in range(2):
                                qrow = slice(par * 64, par * 64 + 64)
                                nc.tensor.matmul(
                                    ps_pair[par][:, lo:hi],
                                    QcT[qrow, dt * Q + it * 128:dt * Q + (it + 1) * 128],
                                    rhT[qrow, dt * KLEN + lo:dt * KLEN + hi],
                                    start=True, stop=True)
                        for par in range(2):
                            # band copy into staging block [it*WBLK, it*WBLK+J)
                            dst = staging[par][:, it * WBLK:it * WBLK + (1024 - jr0)]
                            if par == 0:
                                nc.vector.tensor_copy(dst, ps_pair[par][:, jr0:KLEN])
                            else:
                                nc.scalar.activation(dst, ps_pair[par][:, jr0:KLEN],
                                                     AF.Copy)
                    for par in range(2):
                        h = 2 * hp + par
                        bdd = bd_dram[h]
                        for it in range(4):
                            J = 640 + 128 * it
                            # band + VMASK pad only (skip the garbage cols)
                            nc.sync.dma_start(
                                bdd[:, it * WBLK:it * WBLK + J + 128],
                                staging[par][:, it * WBLK:it * WBLK + J + 128])
                        for it in range(4):
                            J = 640 + 128 * it
                            shifted = bass.AP(bdd[:].tensor, it * WBLK + 127,
                                              [[4 * WBLK - 1, 128], [1, J]])
                            nc.sync.dma_start(
                                out=scratch[h][:, SOFF[it]:SOFF[it] + J],
                                in_=shifted)

                # ---- V projection on RAW kv; mean-correction as rank-1
                # accumulation, then V^T = ps_v * rho_t in one strided op
                for jt in range(8):
                    ps_v = PSP2.tile([128, 512], F32, tag="ps512", name=f"psv{jt}")
                    for kt in range(8):
                        nc.tensor.matmul(
                            ps_v[:],
                            xhat[:, kt * KLEN + jt * 128:kt * KLEN + (jt + 1) * 128],
                            Wv_sb[:, kt * 512:(kt + 1) * 512],
                            start=(kt == 0), stop=False,
                            skip_group_check=True)
                    nc.tensor.matmul(
                        ps_v[:], mu_kv[0:1, jt * 128:(jt + 1) * 128],
                        vsum_sb[0:1, :], start=False, stop=True,
                        skip_group_check=True)
                    vdst = bass.AP(V_sb[:].tensor, V_sb[:].offset + jt * 520,
                                   [[V_sb[:].ap[0][0], 128], [65, 8], [1, 64]])
                    nc.vector.tensor_scalar_mul(
                        vdst, ps_v[:].rearrange("p (h c) -> p h c", h=8),
                        rv_sb[:, jt:jt + 1])
                dump_tile(2, KT_sb[:, 0:1024])
                dump_tile(3, V_sb[:, 0:1024])

                # dummy collective: absorb ncfw entry cost well before the
                # real one (gpsimd queue is idle after the V fixups)
                nc.gpsimd.collective_compute(
                    "AllReduce", mybir.AluOpType.add,
                    replica_groups=[[0, 1], [2, 3], [4, 5], [6, 7]],
                    ins=[ccd_in[:]], outs=[ccd_out[:]])
            LNS.__exit__(None, None, None)
            SPANW.__exit__(None, None, None)
            SPAN.__exit__(None, None, None)

            # ====== phase D: attention + fused output transposes ======
            inter_bf = P.tile([128, 2 * 512], BF, tag="interbf")
            with tc.tile_pool(name="psJ", bufs=1, space="PSUM") as PSJ:
                ps_int = [PSJ.tile([128, 512], F32, tag=f"psint{mt}",
                                   name=f"psint{mt}")
                          for mt in range(2)]
                with (
                    tc.tile_pool(name="epool", bufs=3) as EP,
                    tc.tile_pool(name="psS", bufs=3, space="PSUM") as PSS,
                    tc.tile_pool(name="psAV", bufs=2, space="PSUM") as PSAV,
                    tc.tile_pool(name="psT", bufs=1, space="PSUM") as PST,
                ):
                    for hp in range(4):
                        heads = (2 * hp, 2 * hp + 1)
                        dt = hp
                        E_tiles = {h: EP.tile([128, 8 * 512], BF, tag="E",
                                              name=f"E{h}") for h in heads}
                        for jt in range(8):
                            i0 = max(0, jt - 4) * 128
                            # both heads' K=64 AC matmuls adjacent -> they
                            # pack onto independent PE row-tiles
                            ps_j = {}
                            for h in heads:
                                qrow = slice((h % 2) * 64, (h % 2) * 64 + 64)
                                ps_s = PSS.tile([128, 512], F32, tag="pss",
                                                name=f"pss{h}{jt}")
                                ps_j[h] = ps_s
                                nc.tensor.matmul(
                                    ps_s[:, i0:512],
                                    KT_sb[qrow, dt * KLEN + jt * 128:dt * KLEN + (jt + 1) * 128],
                                    QbT[qrow, dt * Q + i0:dt * Q + 512],
                                    start=True, stop=False, skip_group_check=True)
                            for h in heads:
                                for ib in range(max(0, jt - 4), 4):
                                    nc.tensor.matmul(
                                        ps_j[h][:, ib * 128:(ib + 1) * 128],
                                        scratch[h][:, SOFF[ib] + jt * 128:
                                                   SOFF[ib] + (jt + 1) * 128],
                                        id_sb[:],
                                        start=False, stop=(ib == 3),
                                        skip_group_check=True)
                                nc.scalar.activation(
                                    E_tiles[h][:, jt * 512 + i0:(jt + 1) * 512],
                                    ps_j[h][:, i0:512], AF.Exp, scale=SCALE)
                        for it in range(4):
                            for h in heads:
                                ps_av = PSAV.tile([128, 65], F32, tag="psav",
                                                  name=f"psav{h}{it}")
                                jts = VALID[it]
                                for idx, jt in enumerate(jts):
                                    nc.tensor.matmul(
                                        ps_av[:],
                                        E_tiles[h][:, jt * 512 + it * 128:
                                                   jt * 512 + (it + 1) * 128],
                                        V_sb[:, jt * 520 + h * 65:jt * 520 + (h + 1) * 65],
                                        start=(idx == 0), stop=(idx == len(jts) - 1))
                                rec = W.tile([128, 1], F32, tag="rec")
                                nc.vector.reciprocal(rec[:], ps_av[:, 64:65])
                                nc.vector.tensor_scalar_mul(
                                    attn_sb[:, it * 512 + h * 64:it * 512 + (h + 1) * 64],
                                    ps_av[:, 0:64], rec[:])
                        # avT transpose for this head pair's channels (dt)
                        ps_t = PST.tile([128, 512], F32, tag="psavt",
                                        name=f"psavt{dt}")
                        for it in range(4):
                            nc.tensor.matmul(
                                ps_t[:, it * 128:(it + 1) * 128],
                                attn_sb[:, it * 512 + dt * 128:it * 512 + (dt + 1) * 128],
                                id_sb[:], start=True, stop=True)
                        nc.vector.tensor_copy(avT_sb[:, dt * 512:(dt + 1) * 512],
                                              ps_t[:])
                        # inter partial accumulation (kt = dt slice of Winter)
                        for mt in range(2):
                            nc.tensor.matmul(
                                ps_int[mt][:],
                                Winter_sb[:, dt * DG + mt * 128:dt * DG + (mt + 1) * 128],
                                avT_sb[:, dt * 512:(dt + 1) * 512],
                                start=(dt == 0), stop=(dt == 3),
                                skip_group_check=True)
                        if hp == 0:
                            dump_tile(6, E_tiles[0][:, 0:1024])
                            dump_tile(7, E_tiles[0][:, 1536:2560])
                SCR.__exit__(None, None, None)

                # ====== phase E: collective + intra + output ======
                with (
                    tc.tile_pool(name="phE", bufs=1) as PE_,
                    tc.tile_pool(name="psI", bufs=1, space="PSUM") as PSI,
                ):
                    for mt in range(2):
                        nc.vector.tensor_copy(inter_bf[:, mt * 512:(mt + 1) * 512],
                                              ps_int[mt][:])
                        nc.sync.dma_start(cc_in[mt * 128:(mt + 1) * 128, :],
                                          inter_bf[:, mt * 512:(mt + 1) * 512])
                    nc.gpsimd.collective_compute(
                        "AllReduce", mybir.AluOpType.add,
                        replica_groups=[[0, 1], [2, 3], [4, 5], [6, 7]],
                        ins=[cc_in[:]], outs=[cc_out[:]])

                    inter_rd = PE_.tile([128, 2 * 512], BF, tag="interrd")
                    wres_sb = PE_.tile([128, 4 * 512], F32, tag="wres")
                    nc.sync.dma_start(
                        wres_sb[:].rearrange("p (t q) -> p t q", t=4),
                        wres[:].rearrange("(t p) q -> p t q", p=128))
                    intra_ps = []
                    for t in range(4):
                        gl, mt = t // 2, t % 2
                        ps_o = PSI.tile([128, 512], F32, tag=f"psintra{t}")
                        for kt in range(2):
                            blk = gl * 2 + kt
                            nc.tensor.matmul(
                                ps_o[:],
                                Wintra_sb[:, blk * DG + mt * 128:blk * DG + (mt + 1) * 128],
                                avT_sb[:, blk * 512:(blk + 1) * 512],
                                start=(kt == 0), stop=(kt == 1))
                        intra_ps.append(ps_o)

                    for mt in range(2):
                        nc.sync.dma_start(inter_rd[:, mt * 512:(mt + 1) * 512],
                                          cc_out[mt * 128:(mt + 1) * 128, :])

                    out_f = PE_.tile([128, 4 * 512], F32, tag="outf")
                    for t in range(4):
                        mt = t % 2
                        sl = slice(t * 512, (t + 1) * 512)
                        msl = slice(mt * 512, (mt + 1) * 512)
                        tf = W.tile([128, 512], F32, tag="tf")
                        nc.vector.tensor_add(tf[:], intra_ps[t][:], inter_rd[:, msl])
                        nc.vector.tensor_add(out_f[:, sl], tf[:], wres_sb[:, sl])
                        nc.sync.dma_start(out[t * 128:(t + 1) * 128, :], out_f[:, sl])

    nc.finalize()
    return nc


def _host_prep(inputs):
    import concourse.mybir as mybir
    bf = mybir.dt.np(mybir.dt.bfloat16)

    f32 = lambda x: np.ascontiguousarray(np.asarray(x, np.float32))
    tobf = lambda x: np.ascontiguousarray(np.asarray(x, np.float32).astype(bf))

    w = f32(inputs["w"])
    r = f32(inputs["r"])
    mems = f32(inputs["mems"])
    gkv, bkv = f32(inputs["gamma_kv"]), f32(inputs["beta_kv"])
    gq, bq = f32(inputs["gamma_q"]), f32(inputs["beta_q"])
    Wk, Wv = f32(inputs["Wk"]), f32(inputs["Wv"])
    Wq_, Wiq = f32(inputs["Wq"]), f32(inputs["Wiq"])
    Wr_ = f32(inputs["Wr"])
    Wintra, Winter = f32(inputs["Wintra"]), f32(inputs["Winter"])
    rwb_full = f32(inputs["r_w_bias"]).reshape(D)
    rrb_full = f32(inputs["r_r_bias"]).reshape(D)
    kv = np.concatenate([mems, w], 0)
    ident = np.eye(128, dtype=np.float32).astype(bf)

    in_maps = []
    for core in range(8):
        b, s = core // 2, core % 2
        CH0 = 512 * s
        g0, g1 = 2 * s, 2 * s + 1
        perm = np.r_[CH0:CH0 + 512, (512 - CH0):(512 - CH0) + 512]

        qbeta_g = Wiq @ bq
        qbeta = np.concatenate([
            Wq_[g0] @ bq[g0 * DG:(g0 + 1) * DG] + qbeta_g,
            Wq_[g1] @ bq[g1 * DG:(g1 + 1) * DG] + qbeta_g])
        Wkp = Wk[CH0:CH0 + 512, :] * gkv[None, :]
        Wvp = Wv[CH0:CH0 + 512, :] * gkv[None, :]
        m = {
            "kvT": tobf(kv[:, b, :].T),
            "wTp": tobf(w[:, b, perm].T),
            "wres": f32(w[:, b, CH0:CH0 + 512].T),
            "rT": tobf(r[:, 0, CH0:CH0 + 512].T),
            "WkT": tobf(Wkp.T),
            "WvT": tobf(Wvp.T),
            "WiqT": tobf((Wiq * gq[None, :]).T[perm, :]),
            "WqT": tobf(np.stack([
                (Wq_[g] * gq[None, g * DG:(g + 1) * DG]).T for g in (g0, g1)])),
            "WrT": tobf(np.stack([Wr_[g].T for g in (g0, g1)])),
            "WintraT": tobf(np.stack([Wintra[g].T for g in (g0, g1)])),
            "WinterT": tobf(Winter[:, CH0:CH0 + 512].T),
            "kbeta": f32(Wk[CH0:CH0 + 512, :] @ bkv).reshape(4, 128).T,
            "ksum": tobf(np.broadcast_to(-Wkp.sum(1)[None, :], (128, 512))),
            "vsum": tobf(np.broadcast_to(-Wvp.sum(1)[None, :], (128, 512))),
            "rwb": f32(rwb_full[CH0:CH0 + 512] + qbeta).reshape(4, 128).T,
            "rrb": f32(rrb_full[CH0:CH0 + 512] + qbeta).reshape(4, 128).T,
            "ident": ident,
        }
        vbeta = Wv[CH0:CH0 + 512, :] @ bkv
        assert np.abs(vbeta).max() < 1e-6, "nonzero beta_kv for V not supported"
        in_maps.append(m)
    return in_maps


def kernel(**inputs):
    from concourse.bass_utils import run_bass_kernel_spmd

    if "nc" not in _cache:
        _cache["nc"] = _build_nc()
    nc = _cache["nc"]
    in_maps = _host_prep(inputs)
    res = run_bass_kernel_spmd(nc, in_maps, core_ids=list(range(8)))
    _cache["last_results"] = res

    full = np.zeros((Q, B, D), np.float32)
    for core in range(8):
        b, s = core // 2, core % 2
        o = np.asarray(res.results[core]["out"], np.float32)   # [512 ch, 512 q]
        full[:, b, 512 * s:512 * s + 512] = o.T
    return full
